# revision 6
# baseline (speedup 1.0000x reference)
"""Trainium2 Bass kernel for BatchedGNNModel (4-layer GCN over 3-rod chain graph).

Contract: kernel(**inputs) takes FULL unsharded inputs (as produced by
setup_inputs) and returns the FULL (64, 768, 3) float32 output.

Strategy (pure data-parallel over batch, 8 NeuronCores):
  - Host: normalize adjacency (A_norm), precompute A_norm^T, (A_norm@A_norm)^T,
    transposed weights, clamp input positions, pack per-core shards.
  - Device (identical SPMD program, different batch shard per core):
      per item:  x^T --feat1(K=6)--> Z1 (node-major)
                 --agg1 (A_norm^T, accumulate over node K-tiles)--> H1 (feat-major, relu)
                 --feat2--> Z2 --agg2--> H2 (relu)
                 --feat34 (W3^T@W4^T fused; L3/L4 have no relu between)--> Z34
      then one packed agg over A2 = A_norm@A_norm for all items -> out^T.
  - Host: gather per-core outputs, transpose, apply output clamp.

Layouts alternate feature-major <-> node-major so no on-device transposes are
needed: feature matmuls contract the feature dim (activations stationary),
aggregations contract the node dim (activations stationary, A^T moving).
"""

import os
import sys

import numpy as np

sys.path.insert(0, "/opt/trn_rl_repo")

import concourse.bass as bass
import concourse.mybir as mybir
import concourse.tile as _tile_mod
from concourse.tile import TileContext
from concourse.vector_clock import ScopedClock
from concourse.bass_utils import run_bass_kernel_spmd


def _patched_drain_and_barrier(self, tick_clock, wait_clock):
    """The nix walrus in this image only supports one sync-wait slot on a
    Drain; Tile's kernel-tail drain carries one wait per ticked semaphore.
    Split the extra waits onto single-wait nops on the same (sync) engine —
    program order makes this equivalent before the all-engine barrier."""
    drain_inst = self.nc.sync.drain()
    wait_clock.add_sem_waits(
        drain_inst.ins, ScopedClock({None: tick_clock.global_clock}))
    waits = list(drain_inst.ins.sync_info.on_wait)
    if len(waits) > 1:
        drain_inst.ins.sync_info.on_wait = [waits[0]]
        for w in waits[1:]:
            import bass_rust
            nop = self.nc.sync.nop(nofuse=True)
            si = nop.ins.sync_info
            if si is None:
                nop.ins.sync_info = bass_rust.SyncInfo(on_wait=[w], on_update=[])
            else:
                si.on_wait = [w]
    self.nc.all_engine_barrier()
    assert self.sems is not None
    popped = self.nc._tile_sem_poison_stack.pop()
    assert popped is self._sem_poison
    self.nc.clear_and_free_semaphores(list(self.sems.allocated().values()))
    self.nc.all_engine_barrier()


_tile_mod.TileContext._drain_and_barrier = _patched_drain_and_barrier


def _split_multi_waits(nc):
    """This image's walrus supports a single sync-wait slot per instruction.
    Hoist all-but-one wait of any multi-wait instruction onto single-wait
    NoOps on the same engine, placed immediately before it (same per-engine
    program order => equivalent synchronization)."""
    for f in nc.m.functions:
        for bb in f.blocks:
            insts = list(bb.instructions)
            if not any(ins.sync_info and len(ins.sync_info.on_wait) > 1
                       for ins in insts):
                continue
            new = []
            for ins in insts:
                si = ins.sync_info
                if si is not None and len(si.on_wait) > 1:
                    waits = list(si.on_wait)
                    for w in waits[:-1]:
                        new.append(mybir.InstNoOp(
                            name=nc.get_next_instruction_name(),
                            sync_info=mybir.SyncInfo(on_wait=[w], on_update=[]),
                            bass_nofuse=True,
                            engine=ins.engine,
                        ))
                    si.on_wait = [waits[-1]]
                new.append(ins)
            bb.instructions = new


def _ensure_ntff_hook():
    """The agent image's antenv lacks axon_hooks; bass_utils imports it when
    trace=True. Install a shim and, if possible, the real ctypes profiler."""
    import types
    try:
        import antenv.axon_hooks  # noqa: F401
        return
    except Exception:
        pass
    try:
        import antenv
        mod = types.ModuleType("antenv.axon_hooks")
        state = {"h": None}
        mod.set_axon_ntff_profile_hook = lambda h: state.__setitem__("h", h)
        mod.get_axon_ntff_profile_hook = lambda: state["h"]
        sys.modules["antenv.axon_hooks"] = mod
        antenv.axon_hooks = mod
        try:
            from trn_agent_boot.trn_boot import _ntff_profile_via_ctypes
            mod.set_axon_ntff_profile_hook(
                _ntff_profile_via_ctypes("/opt/axon/libaxon_pjrt.so"))
        except Exception:
            pass
    except Exception:
        pass


_ensure_ntff_hook()

F32 = mybir.dt.float32
RELU = mybir.ActivationFunctionType.Relu

B = 64
NV = 256
N = 3 * NV  # 768
NCORES = 8
IPC = B // NCORES  # 8 items per core
KT = N // 128      # 6 node K-tiles

LAST_RUN_INFO = {}


def _build_program(with_bias: bool):
    nc = bass.Bass()

    xT_d = nc.declare_dram_parameter("xT", [IPC, 6, N], F32, isOutput=False)
    anT_d = nc.declare_dram_parameter("anT", [N, N], F32, isOutput=False)
    a2T_d = nc.declare_dram_parameter("a2T", [N, N], F32, isOutput=False)
    w1T_d = nc.declare_dram_parameter("w1T", [6, 256], F32, isOutput=False)
    w2Tp_d = nc.declare_dram_parameter("w2Tp", [128, 256], F32, isOutput=False)
    w34T_d = nc.declare_dram_parameter("w34T", [128, 3], F32, isOutput=False)
    if with_bias:
        p1t_d = nc.declare_dram_parameter("p1t", [128, 2 * N], F32, isOutput=False)
        p2t_d = nc.declare_dram_parameter("p2t", [128, N], F32, isOutput=False)
        cpt_d = nc.declare_dram_parameter("cpt", [3 * IPC, N], F32, isOutput=False)
    out_d = nc.declare_dram_parameter("outp", [3 * IPC, N], F32, isOutput=True)

    with TileContext(nc) as tc:
        with (
            tc.tile_pool(name="const", bufs=1) as cpool,
            tc.tile_pool(name="acts", bufs=2) as apool,
            tc.tile_pool(name="psf", bufs=2, space="PSUM") as psf,
            tc.tile_pool(name="psa", bufs=3, space="PSUM") as psa,
        ):
            anT = cpool.tile([128, KT * N], F32)  # [p, k*768 + j]
            nc.sync.dma_start(
                anT[:, :].rearrange("p (k j) -> p k j", j=N),
                anT_d[:, :].rearrange("(k p) j -> p k j", p=128))
            a2T = cpool.tile([128, KT * N], F32)
            nc.sync.dma_start(
                a2T[:, :].rearrange("p (k j) -> p k j", j=N),
                a2T_d[:, :].rearrange("(k p) j -> p k j", p=128))
            w1T = cpool.tile([6, 256], F32)
            nc.sync.dma_start(w1T[:, :], w1T_d[:, :])
            w2Tp = cpool.tile([128, 256], F32)
            nc.sync.dma_start(w2Tp[:, :], w2Tp_d[:, :])
            w34T = cpool.tile([128, 3], F32)
            nc.sync.dma_start(w34T[:, :], w34T_d[:, :])
            if with_bias:
                p1t = cpool.tile([128, 2 * N], F32)
                nc.sync.dma_start(p1t[:, :], p1t_d[:, :])
                p2t = cpool.tile([128, N], F32)
                nc.sync.dma_start(p2t[:, :], p2t_d[:, :])
                cpt = cpool.tile([3 * IPC, N], F32)
                nc.sync.dma_start(cpt[:, :], cpt_d[:, :])

            # Z34 for all items: [p, k*3*IPC + it*3 + f]
            z34 = cpool.tile([128, KT * 3 * IPC], F32)

            for it in range(IPC):
                xT = apool.tile([6, N], F32, tag="xT")
                nc.sync.dma_start(xT[:, :], xT_d[it])

                # feat1: Z1[node, fo] = sum_fi xT[fi, node] * W1T[fi, fo]
                z1 = apool.tile([128, KT * 256], F32, tag="z1")  # [p, m*256 + fo]
                for m in range(KT):
                    ps = psf.tile([128, 256], F32, tag="feat")
                    nc.tensor.matmul(
                        ps[:, :], xT[:, m * 128:(m + 1) * 128], w1T[:, :],
                        start=True, stop=True,
                    )
                    nc.vector.tensor_copy(z1[:, m * 256:(m + 1) * 256], ps[:, :])

                # agg1: H1t[f, j] = relu(sum_k Z1[k, f] * AnT[k, j] (+ s x b1))
                h1t = apool.tile([128, 2 * N], F32, tag="h1t")  # [fi, fh*768 + n]
                for fh in range(2):
                    for ns in range(2):
                        ps = psa.tile([128, 384], F32, tag="agg")
                        for k in range(KT):
                            nc.tensor.matmul(
                                ps[:, :],
                                z1[:, k * 256 + fh * 128: k * 256 + fh * 128 + 128],
                                anT[:, k * N + ns * 384: k * N + ns * 384 + 384],
                                start=(k == 0), stop=(k == KT - 1),
                            )
                        dst = h1t[:, fh * N + ns * 384: fh * N + ns * 384 + 384]
                        if with_bias:
                            nc.vector.tensor_tensor(
                                dst, ps[:, :],
                                p1t[:, fh * N + ns * 384: fh * N + ns * 384 + 384],
                                op=mybir.AluOpType.add,
                            )
                            nc.scalar.activation(dst, dst, RELU)
                        else:
                            nc.scalar.activation(dst, ps[:, :], RELU)

                # feat2: Z2[node, fo] = sum_fi H1t[fi, node] * W2T[fi, fo]
                z2 = apool.tile([128, KT * 128], F32, tag="z2")  # [p, m*128 + fo]
                for m in range(KT):
                    ps = psf.tile([128, 128], F32, tag="feat")
                    for kh in range(2):
                        nc.tensor.matmul(
                            ps[:, :],
                            h1t[:, kh * N + m * 128: kh * N + m * 128 + 128],
                            w2Tp[:, kh * 128:(kh + 1) * 128],
                            start=(kh == 0), stop=(kh == 1),
                        )
                    nc.vector.tensor_copy(z2[:, m * 128:(m + 1) * 128], ps[:, :])

                # agg2 + relu -> H2t (feature-major, 128 x 768)
                h2t = apool.tile([128, N], F32, tag="h2t")
                for ns in range(2):
                    ps = psa.tile([128, 384], F32, tag="agg")
                    for k in range(KT):
                        nc.tensor.matmul(
                            ps[:, :],
                            z2[:, k * 128:(k + 1) * 128],
                            anT[:, k * N + ns * 384: k * N + ns * 384 + 384],
                            start=(k == 0), stop=(k == KT - 1),
                        )
                    dst = h2t[:, ns * 384: ns * 384 + 384]
                    if with_bias:
                        nc.vector.tensor_tensor(
                            dst, ps[:, :], p2t[:, ns * 384: ns * 384 + 384],
                            op=mybir.AluOpType.add,
                        )
                        nc.scalar.activation(dst, dst, RELU)
                    else:
                        nc.scalar.activation(dst, ps[:, :], RELU)

                # feat34: Z34[node, f] = sum_fi H2t[fi, node] * W34T[fi, f]
                for m in range(KT):
                    ps = psf.tile([128, 3], F32, tag="feat")
                    nc.tensor.matmul(
                        ps[:, :], h2t[:, m * 128:(m + 1) * 128], w34T[:, :],
                        start=True, stop=True,
                    )
                    base = m * 3 * IPC + it * 3
                    nc.vector.tensor_copy(z34[:, base: base + 3], ps[:, :])

            # final aggregation with A2 for all items at once
            outT = cpool.tile([3 * IPC, N], F32)
            for ns in range(2):
                ps = psa.tile([3 * IPC, 384], F32, tag="agg")
                for k in range(KT):
                    nc.tensor.matmul(
                        ps[:, :],
                        z34[:, k * 3 * IPC:(k + 1) * 3 * IPC],
                        a2T[:, k * N + ns * 384: k * N + ns * 384 + 384],
                        start=(k == 0), stop=(k == KT - 1),
                    )
                dst = outT[:, ns * 384: ns * 384 + 384]
                if with_bias:
                    nc.vector.tensor_tensor(
                        dst, ps[:, :], cpt[:, ns * 384: ns * 384 + 384],
                        op=mybir.AluOpType.add,
                    )
                else:
                    nc.vector.tensor_copy(dst, ps[:, :])
            nc.sync.dma_start(out_d[:, :], outT[:, :])

    return nc


def kernel(x, inputs, adjacency, W1, b1, W2, b2, W3, b3, W4, b4,
           parent_sel, child1_sel, child2_sel):
    global LAST_RUN_INFO
    x = np.asarray(x, np.float32)
    inp = np.asarray(inputs, np.float32)
    A = np.asarray(adjacency, np.float32)
    W1 = np.asarray(W1, np.float32); b1 = np.asarray(b1, np.float32)
    W2 = np.asarray(W2, np.float32); b2 = np.asarray(b2, np.float32)
    W3 = np.asarray(W3, np.float32); b3 = np.asarray(b3, np.float32)
    W4 = np.asarray(W4, np.float32); b4 = np.asarray(b4, np.float32)
    parent_sel = np.asarray(parent_sel, np.int64)
    child1_sel = np.asarray(child1_sel, np.int64)
    child2_sel = np.asarray(child2_sel, np.int64)

    # ---- host prep (replicated constants + layout marshaling) ----
    # clamp rows in global node index space
    clamp_rows = np.concatenate([
        parent_sel, NV + child1_sel, 2 * NV + child2_sel,
    ]).astype(np.int64)

    x0 = x.copy()
    x0[:, clamp_rows, 0:3] = inp[:, clamp_rows, :]

    deg = A.sum(axis=-1)
    deg_safe = np.where(deg == 0, np.float32(1.0), deg)
    d = np.where(deg == 0, np.float32(0.0), deg_safe ** np.float32(-0.5)).astype(np.float32)
    A_norm = (A * d[:, None] * d[None, :]).astype(np.float32)
    AnT = np.ascontiguousarray(A_norm.T)
    A2T = np.ascontiguousarray((A_norm @ A_norm).T.astype(np.float32))

    W1T = np.ascontiguousarray(W1.T)                       # (6, 256)
    W2Tp = np.ascontiguousarray(                           # (128, 256): [p, kh*128+f]
        W2.T.reshape(2, 128, 128).transpose(1, 0, 2).reshape(128, 256))
    W34T = np.ascontiguousarray(W3.T @ W4.T)               # (128, 3)

    with_bias = bool(np.any(b1) or np.any(b2) or np.any(b3) or np.any(b4))
    extra = {}
    if with_bias:
        s = A_norm.sum(axis=1).astype(np.float32)          # A_norm @ 1
        s2 = (A_norm @ s).astype(np.float32)
        # P1t[fi, fh*768 + n] = b1[fh*128+fi] * s[n]
        p1t = np.einsum('f,n->fn', b1, s).astype(np.float32)        # (256, 768)
        p1t = p1t.reshape(2, 128, N).transpose(1, 0, 2).reshape(128, 2 * N)
        p2t = np.einsum('f,n->fn', b2, s).astype(np.float32)        # (128, 768)
        cp = (np.einsum('f,n->fn', W4 @ b3, s2) +
              np.einsum('f,n->fn', b4, s)).astype(np.float32)       # (3, 768)
        cpt = np.tile(cp, (IPC, 1)).astype(np.float32)              # (24, 768)
        extra = {"p1t": np.ascontiguousarray(p1t),
                 "p2t": np.ascontiguousarray(p2t),
                 "cpt": np.ascontiguousarray(cpt)}

    # per-core input shards: xT[core][it] = x0[core*IPC+it].T  (6, 768)
    xT_all = np.ascontiguousarray(
        x0.transpose(0, 2, 1).reshape(NCORES, IPC, 6, N))

    nc = _build_program(with_bias)
    _split_multi_waits(nc)

    in_maps = []
    for c in range(NCORES):
        m = {
            "xT": xT_all[c],
            "anT": AnT,
            "a2T": A2T,
            "w1T": W1T,
            "w2Tp": W2Tp,
            "w34T": W34T,
        }
        m.update(extra)
        in_maps.append(m)

    trace = os.environ.get("KERNEL_TRACE", "") == "1"
    res = run_bass_kernel_spmd(nc, in_maps, list(range(NCORES)), trace=trace)

    LAST_RUN_INFO = {
        "exec_time_ns": res.exec_time_ns,
        "mean_exec_time_ns": res.mean_exec_time_ns,
        "max_exec_time_core_id": res.max_exec_time_core_id,
    }

    out = np.empty((B, N, 3), np.float32)
    for c in range(NCORES):
        o = res.results[c]["outp"]  # (24, 768)
        for it in range(IPC):
            out[c * IPC + it] = o[it * 3:(it + 1) * 3, :].T
    # output clamp
    out[:, clamp_rows, :] = inp[:, clamp_rows, :]
    return out


# revision 12
# speedup vs baseline: 1.7039x; 1.7039x over previous
"""Trainium2 Bass kernel for BatchedGNNModel (4-layer GCN over 3-rod chain graph).

Contract: kernel(**inputs) takes FULL unsharded inputs (as produced by
setup_inputs) and returns the FULL (64, 768, 3) float32 output.

Strategy (pure data-parallel over batch, 8 NeuronCores):
  - Host: normalize adjacency (A_norm), precompute A_norm^T, (A_norm@A_norm)^T,
    transposed weights, clamp input positions, pack per-core shards.
  - Device (identical SPMD program, different batch shard per core):
      per item:  x^T --feat1(K=6)--> Z1 (node-major)
                 --agg1 (A_norm^T, accumulate over node K-tiles)--> H1 (feat-major, relu)
                 --feat2--> Z2 --agg2--> H2 (relu)
                 --feat34 (W3^T@W4^T fused; L3/L4 have no relu between)--> Z34
      then one packed agg over A2 = A_norm@A_norm for all items -> out^T.
  - Host: gather per-core outputs, transpose, apply output clamp.

Layouts alternate feature-major <-> node-major so no on-device transposes are
needed: feature matmuls contract the feature dim (activations stationary),
aggregations contract the node dim (activations stationary, A^T moving).
"""

import os
import sys

import numpy as np

sys.path.insert(0, "/opt/trn_rl_repo")

import concourse.bass as bass
import concourse.mybir as mybir
import concourse.tile as _tile_mod
from concourse.tile import TileContext
from concourse.vector_clock import ScopedClock
from concourse.bass_utils import run_bass_kernel_spmd


def _patched_drain_and_barrier(self, tick_clock, wait_clock):
    """The nix walrus in this image only supports one sync-wait slot on a
    Drain; Tile's kernel-tail drain carries one wait per ticked semaphore.
    Split the extra waits onto single-wait nops on the same (sync) engine —
    program order makes this equivalent before the all-engine barrier."""
    drain_inst = self.nc.sync.drain()
    wait_clock.add_sem_waits(
        drain_inst.ins, ScopedClock({None: tick_clock.global_clock}))
    waits = list(drain_inst.ins.sync_info.on_wait)
    if len(waits) > 1:
        drain_inst.ins.sync_info.on_wait = [waits[0]]
        for w in waits[1:]:
            import bass_rust
            nop = self.nc.sync.nop(nofuse=True)
            si = nop.ins.sync_info
            if si is None:
                nop.ins.sync_info = bass_rust.SyncInfo(on_wait=[w], on_update=[])
            else:
                si.on_wait = [w]
    self.nc.all_engine_barrier()
    assert self.sems is not None
    popped = self.nc._tile_sem_poison_stack.pop()
    assert popped is self._sem_poison
    self.nc.clear_and_free_semaphores(list(self.sems.allocated().values()))
    self.nc.all_engine_barrier()


_tile_mod.TileContext._drain_and_barrier = _patched_drain_and_barrier


def _split_multi_waits(nc):
    """This image's walrus supports a single sync-wait slot per instruction.
    Hoist all-but-one wait of any multi-wait instruction onto single-wait
    NoOps on the same engine, placed immediately before it (same per-engine
    program order => equivalent synchronization)."""
    for f in nc.m.functions:
        for bb in f.blocks:
            insts = list(bb.instructions)
            if not any(ins.sync_info and len(ins.sync_info.on_wait) > 1
                       for ins in insts):
                continue
            new = []
            for ins in insts:
                si = ins.sync_info
                if si is not None and len(si.on_wait) > 1:
                    waits = list(si.on_wait)
                    for w in waits[:-1]:
                        new.append(mybir.InstNoOp(
                            name=nc.get_next_instruction_name(),
                            sync_info=mybir.SyncInfo(on_wait=[w], on_update=[]),
                            bass_nofuse=True,
                            engine=ins.engine,
                        ))
                    si.on_wait = [waits[-1]]
                new.append(ins)
            bb.instructions = new


def _ensure_ntff_hook():
    """The agent image's antenv lacks axon_hooks; bass_utils imports it when
    trace=True. Install a shim and, if possible, the real ctypes profiler."""
    import types
    try:
        import antenv.axon_hooks  # noqa: F401
        return
    except Exception:
        pass
    try:
        import antenv
        mod = types.ModuleType("antenv.axon_hooks")
        state = {"h": None}
        mod.set_axon_ntff_profile_hook = lambda h: state.__setitem__("h", h)
        mod.get_axon_ntff_profile_hook = lambda: state["h"]
        sys.modules["antenv.axon_hooks"] = mod
        antenv.axon_hooks = mod
        try:
            from trn_agent_boot.trn_boot import _ntff_profile_via_ctypes
            mod.set_axon_ntff_profile_hook(
                _ntff_profile_via_ctypes("/opt/axon/libaxon_pjrt.so"))
        except Exception:
            pass
    except Exception:
        pass


_ensure_ntff_hook()

F32 = mybir.dt.float32
RELU = mybir.ActivationFunctionType.Relu

B = 64
NV = 256
N = 3 * NV  # 768
NCORES = 8
IPC = B // NCORES  # 8 items per core
KT = N // 128      # 6 node K-tiles

LAST_RUN_INFO = {}


def _build_program(with_bias: bool):
    nc = bass.Bass()

    xT_d = nc.declare_dram_parameter("xT", [IPC, 6, N], F32, isOutput=False)
    anT_d = nc.declare_dram_parameter("anT", [N, N], F32, isOutput=False)
    a2T_d = nc.declare_dram_parameter("a2T", [N, N], F32, isOutput=False)
    w1T_d = nc.declare_dram_parameter("w1T", [6, 256], F32, isOutput=False)
    w2Tp_d = nc.declare_dram_parameter("w2Tp", [128, 256], F32, isOutput=False)
    w34T_d = nc.declare_dram_parameter("w34T", [128, 3], F32, isOutput=False)
    if with_bias:
        p1t_d = nc.declare_dram_parameter("p1t", [128, 2 * N], F32, isOutput=False)
        p2t_d = nc.declare_dram_parameter("p2t", [128, N], F32, isOutput=False)
        cpt_d = nc.declare_dram_parameter("cpt", [3 * IPC, N], F32, isOutput=False)
    out_d = nc.declare_dram_parameter("outp", [3 * IPC, N], F32, isOutput=True)

    with TileContext(nc) as tc:
        with (
            tc.tile_pool(name="const", bufs=1) as cpool,
            tc.tile_pool(name="acts", bufs=2) as apool,
            tc.tile_pool(name="psf", bufs=2, space="PSUM") as psf,
            tc.tile_pool(name="psa", bufs=3, space="PSUM") as psa,
        ):
            anT = cpool.tile([128, KT * N], F32)  # [p, k*768 + j]
            nc.sync.dma_start(
                anT[:, :].rearrange("p (k j) -> p k j", j=N),
                anT_d[:, :].rearrange("(k p) j -> p k j", p=128))
            a2T = cpool.tile([128, KT * N], F32)
            nc.sync.dma_start(
                a2T[:, :].rearrange("p (k j) -> p k j", j=N),
                a2T_d[:, :].rearrange("(k p) j -> p k j", p=128))
            w1T = cpool.tile([6, 256], F32)
            nc.sync.dma_start(w1T[:, :], w1T_d[:, :])
            w2Tp = cpool.tile([128, 256], F32)
            nc.sync.dma_start(w2Tp[:, :], w2Tp_d[:, :])
            w34T = cpool.tile([128, 3], F32)
            nc.sync.dma_start(w34T[:, :], w34T_d[:, :])
            if with_bias:
                p1t = cpool.tile([128, 2 * N], F32)
                nc.sync.dma_start(p1t[:, :], p1t_d[:, :])
                p2t = cpool.tile([128, N], F32)
                nc.sync.dma_start(p2t[:, :], p2t_d[:, :])
                cpt = cpool.tile([3 * IPC, N], F32)
                nc.sync.dma_start(cpt[:, :], cpt_d[:, :])

            # Z34 for all items: [p, k*3*IPC + it*3 + f]
            z34 = cpool.tile([128, KT * 3 * IPC], F32)

            for it in range(IPC):
                xT = apool.tile([6, N], F32, tag="xT")
                nc.sync.dma_start(xT[:, :], xT_d[it])

                # feat1: Z1[node, fo] = sum_fi xT[fi, node] * W1T[fi, fo]
                z1 = apool.tile([128, KT * 256], F32, tag="z1")  # [p, m*256 + fo]
                for m in range(KT):
                    ps = psf.tile([128, 256], F32, tag="feat")
                    nc.tensor.matmul(
                        ps[:, :], xT[:, m * 128:(m + 1) * 128], w1T[:, :],
                        start=True, stop=True,
                    )
                    nc.vector.tensor_copy(z1[:, m * 256:(m + 1) * 256], ps[:, :])

                # agg1: H1t[f, j] = relu(sum_k Z1[k, f] * AnT[k, j] (+ s x b1))
                h1t = apool.tile([128, 2 * N], F32, tag="h1t")  # [fi, fh*768 + n]
                for fh in range(2):
                    for ns in range(2):
                        ps = psa.tile([128, 384], F32, tag="agg")
                        for k in range(KT):
                            nc.tensor.matmul(
                                ps[:, :],
                                z1[:, k * 256 + fh * 128: k * 256 + fh * 128 + 128],
                                anT[:, k * N + ns * 384: k * N + ns * 384 + 384],
                                start=(k == 0), stop=(k == KT - 1),
                            )
                        dst = h1t[:, fh * N + ns * 384: fh * N + ns * 384 + 384]
                        if with_bias:
                            nc.vector.tensor_tensor(
                                dst, ps[:, :],
                                p1t[:, fh * N + ns * 384: fh * N + ns * 384 + 384],
                                op=mybir.AluOpType.add,
                            )
                            nc.scalar.activation(dst, dst, RELU)
                        else:
                            nc.scalar.activation(dst, ps[:, :], RELU)

                # feat2: Z2[node, fo] = sum_fi H1t[fi, node] * W2T[fi, fo]
                z2 = apool.tile([128, KT * 128], F32, tag="z2")  # [p, m*128 + fo]
                for m in range(KT):
                    ps = psf.tile([128, 128], F32, tag="feat")
                    for kh in range(2):
                        nc.tensor.matmul(
                            ps[:, :],
                            h1t[:, kh * N + m * 128: kh * N + m * 128 + 128],
                            w2Tp[:, kh * 128:(kh + 1) * 128],
                            start=(kh == 0), stop=(kh == 1),
                        )
                    nc.vector.tensor_copy(z2[:, m * 128:(m + 1) * 128], ps[:, :])

                # agg2 + relu -> H2t (feature-major, 128 x 768)
                h2t = apool.tile([128, N], F32, tag="h2t")
                for ns in range(2):
                    ps = psa.tile([128, 384], F32, tag="agg")
                    for k in range(KT):
                        nc.tensor.matmul(
                            ps[:, :],
                            z2[:, k * 128:(k + 1) * 128],
                            anT[:, k * N + ns * 384: k * N + ns * 384 + 384],
                            start=(k == 0), stop=(k == KT - 1),
                        )
                    dst = h2t[:, ns * 384: ns * 384 + 384]
                    if with_bias:
                        nc.vector.tensor_tensor(
                            dst, ps[:, :], p2t[:, ns * 384: ns * 384 + 384],
                            op=mybir.AluOpType.add,
                        )
                        nc.scalar.activation(dst, dst, RELU)
                    else:
                        nc.scalar.activation(dst, ps[:, :], RELU)

                # feat34: Z34[node, f] = sum_fi H2t[fi, node] * W34T[fi, f]
                for m in range(KT):
                    ps = psf.tile([128, 3], F32, tag="feat")
                    nc.tensor.matmul(
                        ps[:, :], h2t[:, m * 128:(m + 1) * 128], w34T[:, :],
                        start=True, stop=True,
                    )
                    base = m * 3 * IPC + it * 3
                    nc.vector.tensor_copy(z34[:, base: base + 3], ps[:, :])

            # final aggregation with A2 for all items at once
            outT = cpool.tile([3 * IPC, N], F32)
            for ns in range(2):
                ps = psa.tile([3 * IPC, 384], F32, tag="agg")
                for k in range(KT):
                    nc.tensor.matmul(
                        ps[:, :],
                        z34[:, k * 3 * IPC:(k + 1) * 3 * IPC],
                        a2T[:, k * N + ns * 384: k * N + ns * 384 + 384],
                        start=(k == 0), stop=(k == KT - 1),
                    )
                dst = outT[:, ns * 384: ns * 384 + 384]
                if with_bias:
                    nc.vector.tensor_tensor(
                        dst, ps[:, :], cpt[:, ns * 384: ns * 384 + 384],
                        op=mybir.AluOpType.add,
                    )
                else:
                    nc.vector.tensor_copy(dst, ps[:, :])
            nc.sync.dma_start(out_d[:, :], outT[:, :])

    return nc


MULT = mybir.AluOpType.mult
ADD = mybir.AluOpType.add


def _build_program_v2(entries):
    """Fast path. All activations feature-major; PE does weights-stationary
    feature matmuls only; aggregation with A_norm runs on the vector engine:
      A_norm = T + E,  T[n,m] = d[n]*d[m] for |n-m|<=1,  E sparse (couplings
      + rod-boundary removals).  A@Z = d.*(U + shift(U) + shift'(U)) + E@Z
    with U = d.*Z, shifts along the free (node) axis, E applied as fused
    (mult,add) ops on strided column slices (batched across items).
    entries: list of (j, k, c) with H[:, j] += c * Z[:, k]."""
    nc = bass.Bass()
    W = IPC * N  # 6144

    xpk_d = nc.declare_dram_parameter("xpk", [2, 128, N], F32, isOutput=False)
    dpl_d = nc.declare_dram_parameter("dpl", [128, N], F32, isOutput=False)
    w1rep_d = nc.declare_dram_parameter("w1rep", [128, 256], F32, isOutput=False)
    w2Tp_d = nc.declare_dram_parameter("w2Tp", [128, 256], F32, isOutput=False)
    w3T_d = nc.declare_dram_parameter("w3T", [128, 128], F32, isOutput=False)
    w4T_d = nc.declare_dram_parameter("w4T", [128, 3], F32, isOutput=False)
    out_d = nc.declare_dram_parameter("outp", [2, 128, N], F32, isOutput=True)

    with TileContext(nc) as tc:
        with (
            tc.tile_pool(name="const", bufs=1) as cpool,
            tc.tile_pool(name="acts", bufs=1) as apool,
            tc.tile_pool(name="ps1", bufs=2, space="PSUM") as ps1,
            tc.tile_pool(name="ps2", bufs=3, space="PSUM") as ps2,
            tc.tile_pool(name="ps4", bufs=2, space="PSUM") as ps4,
        ):
            dpl = cpool.tile([128, N], F32)
            nc.sync.dma_start(dpl[:, :], dpl_d[:, :])
            w1rep = cpool.tile([128, 256], F32)
            nc.sync.dma_start(w1rep[:, :], w1rep_d[:, :])
            w2Tp = cpool.tile([128, 256], F32)
            nc.sync.dma_start(w2Tp[:, :], w2Tp_d[:, :])
            w3T = cpool.tile([128, 128], F32)
            nc.sync.dma_start(w3T[:, :], w3T_d[:, :])
            w4T = cpool.tile([128, 3], F32)
            nc.sync.dma_start(w4T[:, :], w4T_d[:, :])

            def tri_narrow(Z, H, U, P):
                """Aggregation for tiles with one item per partition row:
                Z,H,U: (P, N). H = d.*(U + shifts) + E@Z."""
                dv = dpl[0:P, :]
                nc.vector.tensor_mul(U[:, :], dv, Z[:, :])
                nc.vector.tensor_add(H[:, 1:N], U[:, 1:N], U[:, 0:N - 1])
                nc.vector.tensor_copy(H[:, 0:1], U[:, 0:1])
                nc.vector.tensor_add(H[:, 0:N - 1], H[:, 0:N - 1], U[:, 1:N])
                nc.vector.tensor_mul(H[:, :], dv, H[:, :])
                for (j, k, c) in entries:
                    nc.vector.scalar_tensor_tensor(
                        H[:, j:j + 1], Z[:, k:k + 1], float(c), H[:, j:j + 1],
                        op0=MULT, op1=ADD)

            # ---- L1: aggregate x (F=6/item, items on partitions) ----
            G = []
            for g in range(2):
                Xg = apool.tile([128, N], F32, tag=f"xg{g}")
                nc.sync.dma_start(Xg[:, :], xpk_d[g])
                Gg = apool.tile([128, N], F32, tag=f"gg{g}")
                Ug = apool.tile([128, N], F32, tag=f"ug{g}")
                tri_narrow(Xg, Gg, Ug, 128)
                G.append(Gg)

            # ---- feat1 (K=6 row-packed, weights stationary) + relu ----
            h1a = apool.tile([128, W], F32, tag="tmpA")
            h1b = apool.tile([128, W], F32, tag="tmpB")
            H1 = [h1a, h1b]
            for half in range(2):
                for g in range(2):
                    for j in range(4):
                        it = g * 4 + j
                        for ns in range(2):
                            ps = ps1.tile([128, 384], F32, tag="f1")
                            nc.tensor.matmul(
                                ps[:, :],
                                w1rep[32 * j:32 * j + 6,
                                      half * 128:(half + 1) * 128],
                                G[g][32 * j:32 * j + 6,
                                     ns * 384:(ns + 1) * 384],
                                start=True, stop=True,
                                tile_position=(32 * j, 0))
                            nc.scalar.activation(
                                H1[half][:, it * N + ns * 384:
                                         it * N + (ns + 1) * 384],
                                ps[:, :], RELU)

            def agg_packed(Zt, Ht, relu):
                """Aggregation for (128, W) tiles with items along free dim."""
                U = apool.tile([128, W], F32, tag="tmpA")
                S = apool.tile([128, W], F32, tag="tmpB")
                d3 = dpl[:, :].rearrange("p (o n) -> p o n", o=1) \
                    .broadcast_to((128, IPC, N))
                z3 = Zt[:, :].rearrange("p (i n) -> p i n", n=N)
                u3 = U[:, :].rearrange("p (i n) -> p i n", n=N)
                s3 = S[:, :].rearrange("p (i n) -> p i n", n=N)
                h3 = Ht[:, :].rearrange("p (i n) -> p i n", n=N)
                nc.vector.tensor_mul(u3, d3, z3)
                nc.vector.tensor_add(S[:, 1:W], U[:, 1:W], U[:, 0:W - 1])
                nc.vector.tensor_copy(S[:, 0:1], U[:, 0:1])
                nc.vector.tensor_add(S[:, 0:W - 1], S[:, 0:W - 1], U[:, 1:W])
                # remove cross-item shift contamination
                nc.vector.scalar_tensor_tensor(
                    s3[:, 1:IPC, 0:1], u3[:, 0:IPC - 1, N - 1:N], -1.0,
                    s3[:, 1:IPC, 0:1], op0=MULT, op1=ADD)
                nc.vector.scalar_tensor_tensor(
                    s3[:, 0:IPC - 1, N - 1:N], u3[:, 1:IPC, 0:1], -1.0,
                    s3[:, 0:IPC - 1, N - 1:N], op0=MULT, op1=ADD)
                nc.vector.tensor_mul(h3, d3, s3)
                for (j, k, c) in entries:
                    nc.vector.scalar_tensor_tensor(
                        h3[:, :, j:j + 1], z3[:, :, k:k + 1], float(c),
                        h3[:, :, j:j + 1], op0=MULT, op1=ADD)
                if relu:
                    nc.scalar.activation(Ht[:, 0:W // 2], Ht[:, 0:W // 2], RELU)
                    nc.scalar.activation(Ht[:, W // 2:W], Ht[:, W // 2:W], RELU)

            # ---- feat2 (256 -> 128) ----
            Z2 = apool.tile([128, W], F32, tag="tagZ")
            for c12 in range(12):
                ps = ps2.tile([128, 512], F32, tag="f2")
                for kh in range(2):
                    nc.tensor.matmul(
                        ps[:, :], w2Tp[:, kh * 128:(kh + 1) * 128],
                        H1[kh][:, c12 * 512:(c12 + 1) * 512],
                        start=(kh == 0), stop=(kh == 1))
                nc.vector.tensor_copy(Z2[:, c12 * 512:(c12 + 1) * 512], ps[:, :])

            H2 = apool.tile([128, W], F32, tag="tagH")
            agg_packed(Z2, H2, relu=True)

            # ---- feat3 (128 -> 128) ----
            Z3 = apool.tile([128, W], F32, tag="tagZ")
            for c12 in range(12):
                ps = ps2.tile([128, 512], F32, tag="f2")
                nc.tensor.matmul(ps[:, :], w3T[:, :],
                                 H2[:, c12 * 512:(c12 + 1) * 512],
                                 start=True, stop=True)
                nc.vector.tensor_copy(Z3[:, c12 * 512:(c12 + 1) * 512], ps[:, :])

            H3 = apool.tile([128, W], F32, tag="tagH")
            agg_packed(Z3, H3, relu=False)

            # ---- feat4 (128 -> 3) + L4 aggregation ----
            # items of a group land at partition offsets {0,32,64,96} of one
            # PSUM tile via column tile_position, 4 concurrent col-group mms
            for g in range(2):
                G4 = apool.tile([128, N], F32, tag=f"g4{g}")
                for ns in range(2):
                    ps = ps4.tile([128, 384], F32, tag="f4")
                    for j in range(4):
                        it = g * 4 + j
                        nc.tensor.matmul(
                            ps[32 * j:32 * j + 3, :], w4T[:, :],
                            H3[:, it * N + ns * 384: it * N + (ns + 1) * 384],
                            start=True, stop=True,
                            tile_position=(0, 32 * j))
                    nc.vector.tensor_copy(
                        G4[:, ns * 384:(ns + 1) * 384], ps[:, :])
                U4 = apool.tile([128, N], F32, tag=f"u4{g}")
                O4 = apool.tile([128, N], F32, tag=f"o4{g}")
                tri_narrow(G4, O4, U4, 128)
                nc.sync.dma_start(out_d[g], O4[:, :])

    return nc


def kernel(x, inputs, adjacency, W1, b1, W2, b2, W3, b3, W4, b4,
           parent_sel, child1_sel, child2_sel):
    global LAST_RUN_INFO
    x = np.asarray(x, np.float32)
    inp = np.asarray(inputs, np.float32)
    A = np.asarray(adjacency, np.float32)
    W1 = np.asarray(W1, np.float32); b1 = np.asarray(b1, np.float32)
    W2 = np.asarray(W2, np.float32); b2 = np.asarray(b2, np.float32)
    W3 = np.asarray(W3, np.float32); b3 = np.asarray(b3, np.float32)
    W4 = np.asarray(W4, np.float32); b4 = np.asarray(b4, np.float32)
    parent_sel = np.asarray(parent_sel, np.int64)
    child1_sel = np.asarray(child1_sel, np.int64)
    child2_sel = np.asarray(child2_sel, np.int64)

    # ---- host prep (replicated constants + layout marshaling) ----
    # clamp rows in global node index space
    clamp_rows = np.concatenate([
        parent_sel, NV + child1_sel, 2 * NV + child2_sel,
    ]).astype(np.int64)

    x0 = x.copy()
    x0[:, clamp_rows, 0:3] = inp[:, clamp_rows, :]

    deg = A.sum(axis=-1)
    deg_safe = np.where(deg == 0, np.float32(1.0), deg)
    d = np.where(deg == 0, np.float32(0.0), deg_safe ** np.float32(-0.5)).astype(np.float32)
    A_norm = (A * d[:, None] * d[None, :]).astype(np.float32)
    AnT = np.ascontiguousarray(A_norm.T)
    A2T = np.ascontiguousarray((A_norm @ A_norm).T.astype(np.float32))

    W1T = np.ascontiguousarray(W1.T)                       # (6, 256)
    W2Tp = np.ascontiguousarray(                           # (128, 256): [p, kh*128+f]
        W2.T.reshape(2, 128, 128).transpose(1, 0, 2).reshape(128, 256))
    W34T = np.ascontiguousarray(W3.T @ W4.T)               # (128, 3)

    with_bias = bool(np.any(b1) or np.any(b2) or np.any(b3) or np.any(b4))
    extra = {}
    if with_bias:
        s = A_norm.sum(axis=1).astype(np.float32)          # A_norm @ 1
        s2 = (A_norm @ s).astype(np.float32)
        # P1t[fi, fh*768 + n] = b1[fh*128+fi] * s[n]
        p1t = np.einsum('f,n->fn', b1, s).astype(np.float32)        # (256, 768)
        p1t = p1t.reshape(2, 128, N).transpose(1, 0, 2).reshape(128, 2 * N)
        p2t = np.einsum('f,n->fn', b2, s).astype(np.float32)        # (128, 768)
        cp = (np.einsum('f,n->fn', W4 @ b3, s2) +
              np.einsum('f,n->fn', b4, s)).astype(np.float32)       # (3, 768)
        cpt = np.tile(cp, (IPC, 1)).astype(np.float32)              # (24, 768)
        extra = {"p1t": np.ascontiguousarray(p1t),
                 "p2t": np.ascontiguousarray(p2t),
                 "cpt": np.ascontiguousarray(cpt)}

    # sparse residual of A_norm vs the tridiagonal d-outer-product model
    E = A_norm.copy()
    idx = np.arange(N)
    for o in (-1, 0, 1):
        n = idx[max(0, -o):N - max(0, o)]
        E[n, n + o] -= (d[n] * d[n + o]).astype(np.float32)
    nz = np.argwhere(E != 0)
    entries = [(int(j), int(k), float(E[j, k])) for j, k in nz]

    use_v2 = (not with_bias) and len(entries) <= 96

    if use_v2:
        # item-packed inputs: 2 groups of 4 items at partition stride 32
        xpk = np.zeros((NCORES, 2, 128, N), np.float32)
        for c in range(NCORES):
            for g in range(2):
                for j in range(4):
                    xpk[c, g, 32 * j:32 * j + 6, :] = \
                        x0[c * IPC + g * 4 + j].T
        dpl = np.ascontiguousarray(
            np.broadcast_to(d, (128, N)).astype(np.float32))
        w1rep = np.zeros((128, 256), np.float32)
        for j in range(4):
            w1rep[32 * j:32 * j + 6, :] = W1T
        w3T = np.ascontiguousarray(W3.T)
        w4T = np.ascontiguousarray(W4.T)

        nc = _build_program_v2(entries)
        _split_multi_waits(nc)
        in_maps = [{
            "xpk": xpk[c], "dpl": dpl, "w1rep": w1rep,
            "w2Tp": W2Tp, "w3T": w3T, "w4T": w4T,
        } for c in range(NCORES)]
    else:
        # per-core input shards: xT[core][it] = x0[core*IPC+it].T  (6, 768)
        xT_all = np.ascontiguousarray(
            x0.transpose(0, 2, 1).reshape(NCORES, IPC, 6, N))

        nc = _build_program(with_bias)
        _split_multi_waits(nc)

        in_maps = []
        for c in range(NCORES):
            m = {
                "xT": xT_all[c],
                "anT": AnT,
                "a2T": A2T,
                "w1T": W1T,
                "w2Tp": W2Tp,
                "w34T": W34T,
            }
            m.update(extra)
            in_maps.append(m)

    trace = os.environ.get("KERNEL_TRACE", "") == "1"
    res = run_bass_kernel_spmd(nc, in_maps, list(range(NCORES)), trace=trace)

    LAST_RUN_INFO = {
        "exec_time_ns": res.exec_time_ns,
        "mean_exec_time_ns": res.mean_exec_time_ns,
        "max_exec_time_core_id": res.max_exec_time_core_id,
    }

    out = np.empty((B, N, 3), np.float32)
    for c in range(NCORES):
        o = res.results[c]["outp"]
        if use_v2:  # (2, 128, 768), item g*4+j at partitions 32j..32j+3
            for g in range(2):
                for j in range(4):
                    out[c * IPC + g * 4 + j] = o[g, 32 * j:32 * j + 3, :].T
        else:       # (24, 768)
            for it in range(IPC):
                out[c * IPC + it] = o[it * 3:(it + 1) * 3, :].T
    # output clamp
    out[:, clamp_rows, :] = inp[:, clamp_rows, :]
    return out


# revision 15
# speedup vs baseline: 1.9557x; 1.1478x over previous
"""Trainium2 Bass kernel for BatchedGNNModel (4-layer GCN over 3-rod chain graph).

Contract: kernel(**inputs) takes FULL unsharded inputs (as produced by
setup_inputs) and returns the FULL (64, 768, 3) float32 output.

Strategy (pure data-parallel over batch, 8 NeuronCores):
  - Host: normalize adjacency (A_norm), precompute A_norm^T, (A_norm@A_norm)^T,
    transposed weights, clamp input positions, pack per-core shards.
  - Device (identical SPMD program, different batch shard per core):
      per item:  x^T --feat1(K=6)--> Z1 (node-major)
                 --agg1 (A_norm^T, accumulate over node K-tiles)--> H1 (feat-major, relu)
                 --feat2--> Z2 --agg2--> H2 (relu)
                 --feat34 (W3^T@W4^T fused; L3/L4 have no relu between)--> Z34
      then one packed agg over A2 = A_norm@A_norm for all items -> out^T.
  - Host: gather per-core outputs, transpose, apply output clamp.

Layouts alternate feature-major <-> node-major so no on-device transposes are
needed: feature matmuls contract the feature dim (activations stationary),
aggregations contract the node dim (activations stationary, A^T moving).
"""

import os
import sys

import numpy as np

sys.path.insert(0, "/opt/trn_rl_repo")

import concourse.bass as bass
import concourse.mybir as mybir
import concourse.tile as _tile_mod
from concourse.tile import TileContext
from concourse.vector_clock import ScopedClock
from concourse.bass_utils import run_bass_kernel_spmd


def _patched_drain_and_barrier(self, tick_clock, wait_clock):
    """The nix walrus in this image only supports one sync-wait slot on a
    Drain; Tile's kernel-tail drain carries one wait per ticked semaphore.
    Split the extra waits onto single-wait nops on the same (sync) engine —
    program order makes this equivalent before the all-engine barrier."""
    drain_inst = self.nc.sync.drain()
    wait_clock.add_sem_waits(
        drain_inst.ins, ScopedClock({None: tick_clock.global_clock}))
    waits = list(drain_inst.ins.sync_info.on_wait)
    if len(waits) > 1:
        drain_inst.ins.sync_info.on_wait = [waits[0]]
        for w in waits[1:]:
            import bass_rust
            nop = self.nc.sync.nop(nofuse=True)
            si = nop.ins.sync_info
            if si is None:
                nop.ins.sync_info = bass_rust.SyncInfo(on_wait=[w], on_update=[])
            else:
                si.on_wait = [w]
    self.nc.all_engine_barrier()
    assert self.sems is not None
    popped = self.nc._tile_sem_poison_stack.pop()
    assert popped is self._sem_poison
    self.nc.clear_and_free_semaphores(list(self.sems.allocated().values()))
    self.nc.all_engine_barrier()


_tile_mod.TileContext._drain_and_barrier = _patched_drain_and_barrier


def _split_multi_waits(nc):
    """This image's walrus supports a single sync-wait slot per instruction.
    Hoist all-but-one wait of any multi-wait instruction onto single-wait
    NoOps on the same engine, placed immediately before it (same per-engine
    program order => equivalent synchronization)."""
    for f in nc.m.functions:
        for bb in f.blocks:
            insts = list(bb.instructions)
            if not any(ins.sync_info and len(ins.sync_info.on_wait) > 1
                       for ins in insts):
                continue
            new = []
            for ins in insts:
                si = ins.sync_info
                if si is not None and len(si.on_wait) > 1:
                    waits = list(si.on_wait)
                    for w in waits[:-1]:
                        new.append(mybir.InstNoOp(
                            name=nc.get_next_instruction_name(),
                            sync_info=mybir.SyncInfo(on_wait=[w], on_update=[]),
                            bass_nofuse=True,
                            engine=ins.engine,
                        ))
                    si.on_wait = [waits[-1]]
                new.append(ins)
            bb.instructions = new


def _ensure_ntff_hook():
    """The agent image's antenv lacks axon_hooks; bass_utils imports it when
    trace=True. Install a shim and, if possible, the real ctypes profiler."""
    import types
    try:
        import antenv.axon_hooks  # noqa: F401
        return
    except Exception:
        pass
    try:
        import antenv
        mod = types.ModuleType("antenv.axon_hooks")
        state = {"h": None}
        mod.set_axon_ntff_profile_hook = lambda h: state.__setitem__("h", h)
        mod.get_axon_ntff_profile_hook = lambda: state["h"]
        sys.modules["antenv.axon_hooks"] = mod
        antenv.axon_hooks = mod
        try:
            from trn_agent_boot.trn_boot import _ntff_profile_via_ctypes
            mod.set_axon_ntff_profile_hook(
                _ntff_profile_via_ctypes("/opt/axon/libaxon_pjrt.so"))
        except Exception:
            pass
    except Exception:
        pass


_ensure_ntff_hook()

F32 = mybir.dt.float32
RELU = mybir.ActivationFunctionType.Relu

B = 64
NV = 256
N = 3 * NV  # 768
NCORES = 8
IPC = B // NCORES  # 8 items per core
KT = N // 128      # 6 node K-tiles

LAST_RUN_INFO = {}


def _build_program(with_bias: bool):
    nc = bass.Bass()

    xT_d = nc.declare_dram_parameter("xT", [IPC, 6, N], F32, isOutput=False)
    anT_d = nc.declare_dram_parameter("anT", [N, N], F32, isOutput=False)
    a2T_d = nc.declare_dram_parameter("a2T", [N, N], F32, isOutput=False)
    w1T_d = nc.declare_dram_parameter("w1T", [6, 256], F32, isOutput=False)
    w2Tp_d = nc.declare_dram_parameter("w2Tp", [128, 256], F32, isOutput=False)
    w34T_d = nc.declare_dram_parameter("w34T", [128, 3], F32, isOutput=False)
    if with_bias:
        p1t_d = nc.declare_dram_parameter("p1t", [128, 2 * N], F32, isOutput=False)
        p2t_d = nc.declare_dram_parameter("p2t", [128, N], F32, isOutput=False)
        cpt_d = nc.declare_dram_parameter("cpt", [3 * IPC, N], F32, isOutput=False)
    out_d = nc.declare_dram_parameter("outp", [3 * IPC, N], F32, isOutput=True)

    with TileContext(nc) as tc:
        with (
            tc.tile_pool(name="const", bufs=1) as cpool,
            tc.tile_pool(name="acts", bufs=2) as apool,
            tc.tile_pool(name="psf", bufs=2, space="PSUM") as psf,
            tc.tile_pool(name="psa", bufs=3, space="PSUM") as psa,
        ):
            anT = cpool.tile([128, KT * N], F32)  # [p, k*768 + j]
            nc.sync.dma_start(
                anT[:, :].rearrange("p (k j) -> p k j", j=N),
                anT_d[:, :].rearrange("(k p) j -> p k j", p=128))
            a2T = cpool.tile([128, KT * N], F32)
            nc.sync.dma_start(
                a2T[:, :].rearrange("p (k j) -> p k j", j=N),
                a2T_d[:, :].rearrange("(k p) j -> p k j", p=128))
            w1T = cpool.tile([6, 256], F32)
            nc.sync.dma_start(w1T[:, :], w1T_d[:, :])
            w2Tp = cpool.tile([128, 256], F32)
            nc.sync.dma_start(w2Tp[:, :], w2Tp_d[:, :])
            w34T = cpool.tile([128, 3], F32)
            nc.sync.dma_start(w34T[:, :], w34T_d[:, :])
            if with_bias:
                p1t = cpool.tile([128, 2 * N], F32)
                nc.sync.dma_start(p1t[:, :], p1t_d[:, :])
                p2t = cpool.tile([128, N], F32)
                nc.sync.dma_start(p2t[:, :], p2t_d[:, :])
                cpt = cpool.tile([3 * IPC, N], F32)
                nc.sync.dma_start(cpt[:, :], cpt_d[:, :])

            # Z34 for all items: [p, k*3*IPC + it*3 + f]
            z34 = cpool.tile([128, KT * 3 * IPC], F32)

            for it in range(IPC):
                xT = apool.tile([6, N], F32, tag="xT")
                nc.sync.dma_start(xT[:, :], xT_d[it])

                # feat1: Z1[node, fo] = sum_fi xT[fi, node] * W1T[fi, fo]
                z1 = apool.tile([128, KT * 256], F32, tag="z1")  # [p, m*256 + fo]
                for m in range(KT):
                    ps = psf.tile([128, 256], F32, tag="feat")
                    nc.tensor.matmul(
                        ps[:, :], xT[:, m * 128:(m + 1) * 128], w1T[:, :],
                        start=True, stop=True,
                    )
                    nc.vector.tensor_copy(z1[:, m * 256:(m + 1) * 256], ps[:, :])

                # agg1: H1t[f, j] = relu(sum_k Z1[k, f] * AnT[k, j] (+ s x b1))
                h1t = apool.tile([128, 2 * N], F32, tag="h1t")  # [fi, fh*768 + n]
                for fh in range(2):
                    for ns in range(2):
                        ps = psa.tile([128, 384], F32, tag="agg")
                        for k in range(KT):
                            nc.tensor.matmul(
                                ps[:, :],
                                z1[:, k * 256 + fh * 128: k * 256 + fh * 128 + 128],
                                anT[:, k * N + ns * 384: k * N + ns * 384 + 384],
                                start=(k == 0), stop=(k == KT - 1),
                            )
                        dst = h1t[:, fh * N + ns * 384: fh * N + ns * 384 + 384]
                        if with_bias:
                            nc.vector.tensor_tensor(
                                dst, ps[:, :],
                                p1t[:, fh * N + ns * 384: fh * N + ns * 384 + 384],
                                op=mybir.AluOpType.add,
                            )
                            nc.scalar.activation(dst, dst, RELU)
                        else:
                            nc.scalar.activation(dst, ps[:, :], RELU)

                # feat2: Z2[node, fo] = sum_fi H1t[fi, node] * W2T[fi, fo]
                z2 = apool.tile([128, KT * 128], F32, tag="z2")  # [p, m*128 + fo]
                for m in range(KT):
                    ps = psf.tile([128, 128], F32, tag="feat")
                    for kh in range(2):
                        nc.tensor.matmul(
                            ps[:, :],
                            h1t[:, kh * N + m * 128: kh * N + m * 128 + 128],
                            w2Tp[:, kh * 128:(kh + 1) * 128],
                            start=(kh == 0), stop=(kh == 1),
                        )
                    nc.vector.tensor_copy(z2[:, m * 128:(m + 1) * 128], ps[:, :])

                # agg2 + relu -> H2t (feature-major, 128 x 768)
                h2t = apool.tile([128, N], F32, tag="h2t")
                for ns in range(2):
                    ps = psa.tile([128, 384], F32, tag="agg")
                    for k in range(KT):
                        nc.tensor.matmul(
                            ps[:, :],
                            z2[:, k * 128:(k + 1) * 128],
                            anT[:, k * N + ns * 384: k * N + ns * 384 + 384],
                            start=(k == 0), stop=(k == KT - 1),
                        )
                    dst = h2t[:, ns * 384: ns * 384 + 384]
                    if with_bias:
                        nc.vector.tensor_tensor(
                            dst, ps[:, :], p2t[:, ns * 384: ns * 384 + 384],
                            op=mybir.AluOpType.add,
                        )
                        nc.scalar.activation(dst, dst, RELU)
                    else:
                        nc.scalar.activation(dst, ps[:, :], RELU)

                # feat34: Z34[node, f] = sum_fi H2t[fi, node] * W34T[fi, f]
                for m in range(KT):
                    ps = psf.tile([128, 3], F32, tag="feat")
                    nc.tensor.matmul(
                        ps[:, :], h2t[:, m * 128:(m + 1) * 128], w34T[:, :],
                        start=True, stop=True,
                    )
                    base = m * 3 * IPC + it * 3
                    nc.vector.tensor_copy(z34[:, base: base + 3], ps[:, :])

            # final aggregation with A2 for all items at once
            outT = cpool.tile([3 * IPC, N], F32)
            for ns in range(2):
                ps = psa.tile([3 * IPC, 384], F32, tag="agg")
                for k in range(KT):
                    nc.tensor.matmul(
                        ps[:, :],
                        z34[:, k * 3 * IPC:(k + 1) * 3 * IPC],
                        a2T[:, k * N + ns * 384: k * N + ns * 384 + 384],
                        start=(k == 0), stop=(k == KT - 1),
                    )
                dst = outT[:, ns * 384: ns * 384 + 384]
                if with_bias:
                    nc.vector.tensor_tensor(
                        dst, ps[:, :], cpt[:, ns * 384: ns * 384 + 384],
                        op=mybir.AluOpType.add,
                    )
                else:
                    nc.vector.tensor_copy(dst, ps[:, :])
            nc.sync.dma_start(out_d[:, :], outT[:, :])

    return nc


MULT = mybir.AluOpType.mult
ADD = mybir.AluOpType.add


def _build_program_v2(ent_l1, ent_mid, ent_out):
    """Fast path. All activations feature-major; PE does weights-stationary
    feature matmuls; aggregation with A_norm runs on the vector engine:
      A_norm = T + E,  T[n,m] = d[n]*d[m] for |n-m|<=1,  E sparse.
    The trailing d-scale of each aggregation is deferred through the next
    feature matmul / relu (a per-node column scale commutes with both, d>=0),
    so each aggregation is 3 full DVE passes:
      U = plane .* Z;  S[n] = U[n-1]+U[n]+U[n+1];  plus sparse E ops.
    ent_*: (j, k, c) lists with coefficients pre-adjusted for the deferral.
    L2..L4 are emitted per 4-item group so PE/DVE/ACT pipeline across groups.
    """
    nc = bass.Bass()
    W = IPC * N  # 6144
    COPYF = mybir.ActivationFunctionType.Copy

    xpk_d = nc.declare_dram_parameter("xpk", [2, 128, N], F32, isOutput=False)
    dpl_d = nc.declare_dram_parameter("dpl", [128, N], F32, isOutput=False)
    dp2_d = nc.declare_dram_parameter("dp2", [128, N], F32, isOutput=False)
    w1rep_d = nc.declare_dram_parameter("w1rep", [128, 256], F32, isOutput=False)
    w2Tp_d = nc.declare_dram_parameter("w2Tp", [128, 256], F32, isOutput=False)
    w3T_d = nc.declare_dram_parameter("w3T", [128, 128], F32, isOutput=False)
    w4T_d = nc.declare_dram_parameter("w4T", [128, 3], F32, isOutput=False)
    out_d = nc.declare_dram_parameter("outp", [2, 128, N], F32, isOutput=True)

    with TileContext(nc) as tc:
        with (
            tc.tile_pool(name="const", bufs=1) as cpool,
            tc.tile_pool(name="acts", bufs=1) as apool,
            tc.tile_pool(name="grp", bufs=2) as gpool,
            tc.tile_pool(name="ps1", bufs=2, space="PSUM") as ps1,
            tc.tile_pool(name="ps2", bufs=4, space="PSUM") as ps2,
            tc.tile_pool(name="ps4", bufs=2, space="PSUM") as ps4,
        ):
            dpl = cpool.tile([128, N], F32)
            nc.sync.dma_start(dpl[:, :], dpl_d[:, :])
            dp2 = cpool.tile([128, N], F32)
            nc.sync.dma_start(dp2[:, :], dp2_d[:, :])
            w1rep = cpool.tile([128, 256], F32)
            nc.sync.dma_start(w1rep[:, :], w1rep_d[:, :])
            w2Tp = cpool.tile([128, 256], F32)
            nc.sync.dma_start(w2Tp[:, :], w2Tp_d[:, :])
            w3T = cpool.tile([128, 128], F32)
            nc.sync.dma_start(w3T[:, :], w3T_d[:, :])
            w4T = cpool.tile([128, 3], F32)
            nc.sync.dma_start(w4T[:, :], w4T_d[:, :])

            def tri(Z, H, U, zb, b, wid, plane, P=128, ubase=None):
                """S-part of one aggregation on flat tiles: windows
                Z[:, zb:], H[:, b:], U[:, u:] of width wid.
                U = plane.*Z;  H[n] = U[n-1]+U[n]+U[n+1] (in-window)."""
                u = b if ubase is None else ubase
                dv = plane[0:P, 0:wid]
                nc.vector.tensor_mul(U[0:P, u:u + wid], dv, Z[0:P, zb:zb + wid])
                nc.vector.tensor_add(H[0:P, b + 1:b + wid],
                                     U[0:P, u + 1:u + wid],
                                     U[0:P, u:u + wid - 1])
                nc.vector.tensor_copy(H[0:P, b:b + 1], U[0:P, u:u + 1])
                nc.vector.tensor_add(H[0:P, b:b + wid - 1],
                                     H[0:P, b:b + wid - 1],
                                     U[0:P, u + 1:u + wid])

            def ent_cols(Z, H, ents, zb=0, b=0, P=128):
                for (j, k, c) in ents:
                    nc.vector.scalar_tensor_tensor(
                        H[0:P, b + j:b + j + 1], Z[0:P, zb + k:zb + k + 1],
                        float(c), H[0:P, b + j:b + j + 1], op0=MULT, op1=ADD)

            def ent_group(Z, H, ents):
                zv = Z[:, :].rearrange("p (i n) -> p i n", n=N)
                hv = H[:, :].rearrange("p (i n) -> p i n", n=N)
                for (j, k, c) in ents:
                    nc.vector.scalar_tensor_tensor(
                        hv[:, :, j:j + 1], zv[:, :, k:k + 1], float(c),
                        hv[:, :, j:j + 1], op0=MULT, op1=ADD)

            # ---- L1: G' = (unscaled) aggregation of x;  true G = d .* G' ----
            G = []
            for g in range(2):
                Xg = apool.tile([128, N], F32, tag=f"xg{g}")
                nc.sync.dma_start(Xg[:, :], xpk_d[g])
                Gg = apool.tile([128, N], F32, tag=f"gg{g}")
                Ug = apool.tile([128, N], F32, tag=f"ug{g}")
                tri(Xg, Gg, Ug, 0, 0, N, dpl)
                ent_cols(Xg, Gg, ent_l1)
                G.append(Gg)

            # ---- feat1 (K=6 row-packed, weights stationary) + relu ----
            h1a = apool.tile([128, W], F32, tag="h1a")
            h1b = apool.tile([128, W], F32, tag="h1b")
            H1 = [h1a, h1b]
            for g in range(2):
                for j in range(4):
                    it = g * 4 + j
                    for half in range(2):
                        for ns in range(2):
                            ps = ps1.tile([128, 384], F32, tag="f1")
                            nc.tensor.matmul(
                                ps[:, :],
                                w1rep[32 * j:32 * j + 6,
                                      half * 128:(half + 1) * 128],
                                G[g][32 * j:32 * j + 6,
                                     ns * 384:(ns + 1) * 384],
                                start=True, stop=True,
                                tile_position=(32 * j, 0))
                            nc.scalar.activation(
                                H1[half][:, it * N + ns * 384:
                                         it * N + (ns + 1) * 384],
                                ps[:, :], RELU)

            for gi in range(2):
                its = [gi * 4 + j for j in range(4)]
                # feat2 (256 -> 128), evac on ACT
                Z2g = gpool.tile([128, 4 * N], F32, tag="tagZ")
                for it in its:
                    w = (it - gi * 4) * N
                    for cs in range(2):
                        ps = ps2.tile([128, 384], F32, tag="f2")
                        for kh in range(2):
                            nc.tensor.matmul(
                                ps[:, :], w2Tp[:, kh * 128:(kh + 1) * 128],
                                H1[kh][:, it * N + cs * 384:
                                       it * N + (cs + 1) * 384],
                                start=(kh == 0), stop=(kh == 1))
                        nc.scalar.activation(
                            Z2g[:, w + cs * 384: w + (cs + 1) * 384],
                            ps[:, :], COPYF)
                # agg2 (deferred scale) + E + relu
                H2g = gpool.tile([128, 4 * N], F32, tag="tagH")
                for j4 in range(4):
                    Ut = gpool.tile([128, N], F32, tag="tagU")
                    tri(Z2g, H2g, Ut, j4 * N, j4 * N, N, dp2, ubase=0)
                ent_group(Z2g, H2g, ent_mid)
                nc.scalar.activation(H2g[:, :], H2g[:, :], RELU)
                # feat3 (128 -> 128)
                Z3g = gpool.tile([128, 4 * N], F32, tag="tagZ")
                for j4 in range(4):
                    w = j4 * N
                    for cs in range(2):
                        ps = ps2.tile([128, 384], F32, tag="f2")
                        nc.tensor.matmul(
                            ps[:, :], w3T[:, :],
                            H2g[:, w + cs * 384: w + (cs + 1) * 384],
                            start=True, stop=True)
                        nc.scalar.activation(
                            Z3g[:, w + cs * 384: w + (cs + 1) * 384],
                            ps[:, :], COPYF)
                # agg3 (deferred scale) + E  (no relu)
                H3g = gpool.tile([128, 4 * N], F32, tag="tagH")
                for j4 in range(4):
                    Ut = gpool.tile([128, N], F32, tag="tagU")
                    tri(Z3g, H3g, Ut, j4 * N, j4 * N, N, dp2, ubase=0)
                ent_group(Z3g, H3g, ent_mid)
                # feat4 (128 -> 3): 4 items at col-groups of one PSUM tile
                G4 = gpool.tile([128, N], F32, tag="g4")
                for ns in range(2):
                    ps = ps4.tile([128, 384], F32, tag="f4")
                    for j in range(4):
                        nc.tensor.matmul(
                            ps[32 * j:32 * j + 3, :], w4T[:, :],
                            H3g[:, j * N + ns * 384: j * N + (ns + 1) * 384],
                            start=True, stop=True,
                            tile_position=(0, 32 * j))
                    nc.vector.tensor_copy(
                        G4[:, ns * 384:(ns + 1) * 384], ps[:, :])
                # L4 aggregation; final scale applied here (dpl), then E
                U4 = gpool.tile([128, N], F32, tag="tagU")
                O4 = gpool.tile([128, N], F32, tag="o4")
                tri(G4, O4, U4, 0, 0, N, dp2, ubase=0)
                nc.vector.tensor_mul(O4[:, :], dpl[:, :], O4[:, :])
                ent_cols(G4, O4, ent_out)
                nc.sync.dma_start(out_d[gi], O4[:, :])

    return nc


def kernel(x, inputs, adjacency, W1, b1, W2, b2, W3, b3, W4, b4,
           parent_sel, child1_sel, child2_sel):
    global LAST_RUN_INFO
    x = np.asarray(x, np.float32)
    inp = np.asarray(inputs, np.float32)
    A = np.asarray(adjacency, np.float32)
    W1 = np.asarray(W1, np.float32); b1 = np.asarray(b1, np.float32)
    W2 = np.asarray(W2, np.float32); b2 = np.asarray(b2, np.float32)
    W3 = np.asarray(W3, np.float32); b3 = np.asarray(b3, np.float32)
    W4 = np.asarray(W4, np.float32); b4 = np.asarray(b4, np.float32)
    parent_sel = np.asarray(parent_sel, np.int64)
    child1_sel = np.asarray(child1_sel, np.int64)
    child2_sel = np.asarray(child2_sel, np.int64)

    # ---- host prep (replicated constants + layout marshaling) ----
    # clamp rows in global node index space
    clamp_rows = np.concatenate([
        parent_sel, NV + child1_sel, 2 * NV + child2_sel,
    ]).astype(np.int64)

    x0 = x.copy()
    x0[:, clamp_rows, 0:3] = inp[:, clamp_rows, :]

    deg = A.sum(axis=-1)
    deg_safe = np.where(deg == 0, np.float32(1.0), deg)
    d = np.where(deg == 0, np.float32(0.0), deg_safe ** np.float32(-0.5)).astype(np.float32)
    A_norm = (A * d[:, None] * d[None, :]).astype(np.float32)
    AnT = np.ascontiguousarray(A_norm.T)
    A2T = np.ascontiguousarray((A_norm @ A_norm).T.astype(np.float32))

    W1T = np.ascontiguousarray(W1.T)                       # (6, 256)
    W2Tp = np.ascontiguousarray(                           # (128, 256): [p, kh*128+f]
        W2.T.reshape(2, 128, 128).transpose(1, 0, 2).reshape(128, 256))
    W34T = np.ascontiguousarray(W3.T @ W4.T)               # (128, 3)

    with_bias = bool(np.any(b1) or np.any(b2) or np.any(b3) or np.any(b4))
    extra = {}
    if with_bias:
        s = A_norm.sum(axis=1).astype(np.float32)          # A_norm @ 1
        s2 = (A_norm @ s).astype(np.float32)
        # P1t[fi, fh*768 + n] = b1[fh*128+fi] * s[n]
        p1t = np.einsum('f,n->fn', b1, s).astype(np.float32)        # (256, 768)
        p1t = p1t.reshape(2, 128, N).transpose(1, 0, 2).reshape(128, 2 * N)
        p2t = np.einsum('f,n->fn', b2, s).astype(np.float32)        # (128, 768)
        cp = (np.einsum('f,n->fn', W4 @ b3, s2) +
              np.einsum('f,n->fn', b4, s)).astype(np.float32)       # (3, 768)
        cpt = np.tile(cp, (IPC, 1)).astype(np.float32)              # (24, 768)
        extra = {"p1t": np.ascontiguousarray(p1t),
                 "p2t": np.ascontiguousarray(p2t),
                 "cpt": np.ascontiguousarray(cpt)}

    # sparse residual of A_norm vs the tridiagonal d-outer-product model
    E = A_norm.copy()
    idx = np.arange(N)
    for o in (-1, 0, 1):
        n = idx[max(0, -o):N - max(0, o)]
        E[n, n + o] -= (d[n] * d[n + o]).astype(np.float32)
    nz = np.argwhere(E != 0)
    entries = [(int(j), int(k), float(E[j, k])) for j, k in nz]

    use_v2 = (not with_bias) and len(entries) <= 96

    if use_v2:
        # item-packed inputs: 2 groups of 4 items at partition stride 32
        xpk = np.zeros((NCORES, 2, 128, N), np.float32)
        for c in range(NCORES):
            for g in range(2):
                for j in range(4):
                    xpk[c, g, 32 * j:32 * j + 6, :] = \
                        x0[c * IPC + g * 4 + j].T
        dpl = np.ascontiguousarray(
            np.broadcast_to(d, (128, N)).astype(np.float32))
        dp2 = np.ascontiguousarray(
            np.broadcast_to((d * d).astype(np.float32), (128, N)))
        w1rep = np.zeros((128, 256), np.float32)
        for j in range(4):
            w1rep[32 * j:32 * j + 6, :] = W1T
        w3T = np.ascontiguousarray(W3.T)
        w4T = np.ascontiguousarray(W4.T)

        # entry coefficients adjusted for the deferred d-scale
        dj = np.where(d == 0, np.float32(1.0), d)
        ent_l1 = [(j, k, c / float(dj[j])) for (j, k, c) in entries]
        ent_mid = [(j, k, c * float(d[k]) / float(dj[j]))
                   for (j, k, c) in entries]
        ent_out = [(j, k, c * float(d[k])) for (j, k, c) in entries]

        nc = _build_program_v2(ent_l1, ent_mid, ent_out)
        _split_multi_waits(nc)
        in_maps = [{
            "xpk": xpk[c], "dpl": dpl, "dp2": dp2, "w1rep": w1rep,
            "w2Tp": W2Tp, "w3T": w3T, "w4T": w4T,
        } for c in range(NCORES)]
    else:
        # per-core input shards: xT[core][it] = x0[core*IPC+it].T  (6, 768)
        xT_all = np.ascontiguousarray(
            x0.transpose(0, 2, 1).reshape(NCORES, IPC, 6, N))

        nc = _build_program(with_bias)
        _split_multi_waits(nc)

        in_maps = []
        for c in range(NCORES):
            m = {
                "xT": xT_all[c],
                "anT": AnT,
                "a2T": A2T,
                "w1T": W1T,
                "w2Tp": W2Tp,
                "w34T": W34T,
            }
            m.update(extra)
            in_maps.append(m)

    trace = os.environ.get("KERNEL_TRACE", "") == "1"
    res = run_bass_kernel_spmd(nc, in_maps, list(range(NCORES)), trace=trace)

    LAST_RUN_INFO = {
        "exec_time_ns": res.exec_time_ns,
        "mean_exec_time_ns": res.mean_exec_time_ns,
        "max_exec_time_core_id": res.max_exec_time_core_id,
    }

    out = np.empty((B, N, 3), np.float32)
    for c in range(NCORES):
        o = res.results[c]["outp"]
        if use_v2:  # (2, 128, 768), item g*4+j at partitions 32j..32j+3
            for g in range(2):
                for j in range(4):
                    out[c * IPC + g * 4 + j] = o[g, 32 * j:32 * j + 3, :].T
        else:       # (24, 768)
            for it in range(IPC):
                out[c * IPC + it] = o[it * 3:(it + 1) * 3, :].T
    # output clamp
    out[:, clamp_rows, :] = inp[:, clamp_rows, :]
    return out


# revision 16
# speedup vs baseline: 2.4355x; 1.2454x over previous
"""Trainium2 Bass kernel for BatchedGNNModel (4-layer GCN over 3-rod chain graph).

Contract: kernel(**inputs) takes FULL unsharded inputs (as produced by
setup_inputs) and returns the FULL (64, 768, 3) float32 output.

Strategy (pure data-parallel over batch, 8 NeuronCores):
  - Host: normalize adjacency (A_norm), precompute A_norm^T, (A_norm@A_norm)^T,
    transposed weights, clamp input positions, pack per-core shards.
  - Device (identical SPMD program, different batch shard per core):
      per item:  x^T --feat1(K=6)--> Z1 (node-major)
                 --agg1 (A_norm^T, accumulate over node K-tiles)--> H1 (feat-major, relu)
                 --feat2--> Z2 --agg2--> H2 (relu)
                 --feat34 (W3^T@W4^T fused; L3/L4 have no relu between)--> Z34
      then one packed agg over A2 = A_norm@A_norm for all items -> out^T.
  - Host: gather per-core outputs, transpose, apply output clamp.

Layouts alternate feature-major <-> node-major so no on-device transposes are
needed: feature matmuls contract the feature dim (activations stationary),
aggregations contract the node dim (activations stationary, A^T moving).
"""

import os
import sys

import numpy as np

sys.path.insert(0, "/opt/trn_rl_repo")

import concourse.bass as bass
import concourse.mybir as mybir
import concourse.tile as _tile_mod
from concourse.tile import TileContext
from concourse.vector_clock import ScopedClock
from concourse.bass_utils import run_bass_kernel_spmd


def _patched_drain_and_barrier(self, tick_clock, wait_clock):
    """The nix walrus in this image only supports one sync-wait slot on a
    Drain; Tile's kernel-tail drain carries one wait per ticked semaphore.
    Split the extra waits onto single-wait nops on the same (sync) engine —
    program order makes this equivalent before the all-engine barrier."""
    drain_inst = self.nc.sync.drain()
    wait_clock.add_sem_waits(
        drain_inst.ins, ScopedClock({None: tick_clock.global_clock}))
    waits = list(drain_inst.ins.sync_info.on_wait)
    if len(waits) > 1:
        drain_inst.ins.sync_info.on_wait = [waits[0]]
        for w in waits[1:]:
            import bass_rust
            nop = self.nc.sync.nop(nofuse=True)
            si = nop.ins.sync_info
            if si is None:
                nop.ins.sync_info = bass_rust.SyncInfo(on_wait=[w], on_update=[])
            else:
                si.on_wait = [w]
    self.nc.all_engine_barrier()
    assert self.sems is not None
    popped = self.nc._tile_sem_poison_stack.pop()
    assert popped is self._sem_poison
    self.nc.clear_and_free_semaphores(list(self.sems.allocated().values()))
    self.nc.all_engine_barrier()


_tile_mod.TileContext._drain_and_barrier = _patched_drain_and_barrier


def _split_multi_waits(nc):
    """This image's walrus supports a single sync-wait slot per instruction.
    Hoist all-but-one wait of any multi-wait instruction onto single-wait
    NoOps on the same engine, placed immediately before it (same per-engine
    program order => equivalent synchronization)."""
    for f in nc.m.functions:
        for bb in f.blocks:
            insts = list(bb.instructions)
            if not any(ins.sync_info and len(ins.sync_info.on_wait) > 1
                       for ins in insts):
                continue
            new = []
            for ins in insts:
                si = ins.sync_info
                if si is not None and len(si.on_wait) > 1:
                    waits = list(si.on_wait)
                    for w in waits[:-1]:
                        new.append(mybir.InstNoOp(
                            name=nc.get_next_instruction_name(),
                            sync_info=mybir.SyncInfo(on_wait=[w], on_update=[]),
                            bass_nofuse=True,
                            engine=ins.engine,
                        ))
                    si.on_wait = [waits[-1]]
                new.append(ins)
            bb.instructions = new


def _ensure_ntff_hook():
    """The agent image's antenv lacks axon_hooks; bass_utils imports it when
    trace=True. Install a shim and, if possible, the real ctypes profiler."""
    import types
    try:
        import antenv.axon_hooks  # noqa: F401
        return
    except Exception:
        pass
    try:
        import antenv
        mod = types.ModuleType("antenv.axon_hooks")
        state = {"h": None}
        mod.set_axon_ntff_profile_hook = lambda h: state.__setitem__("h", h)
        mod.get_axon_ntff_profile_hook = lambda: state["h"]
        sys.modules["antenv.axon_hooks"] = mod
        antenv.axon_hooks = mod
        try:
            from trn_agent_boot.trn_boot import _ntff_profile_via_ctypes
            mod.set_axon_ntff_profile_hook(
                _ntff_profile_via_ctypes("/opt/axon/libaxon_pjrt.so"))
        except Exception:
            pass
    except Exception:
        pass


_ensure_ntff_hook()

F32 = mybir.dt.float32
RELU = mybir.ActivationFunctionType.Relu

B = 64
NV = 256
N = 3 * NV  # 768
NCORES = 8
IPC = B // NCORES  # 8 items per core
KT = N // 128      # 6 node K-tiles

LAST_RUN_INFO = {}


def _build_program(with_bias: bool):
    nc = bass.Bass()

    xT_d = nc.declare_dram_parameter("xT", [IPC, 6, N], F32, isOutput=False)
    anT_d = nc.declare_dram_parameter("anT", [N, N], F32, isOutput=False)
    a2T_d = nc.declare_dram_parameter("a2T", [N, N], F32, isOutput=False)
    w1T_d = nc.declare_dram_parameter("w1T", [6, 256], F32, isOutput=False)
    w2Tp_d = nc.declare_dram_parameter("w2Tp", [128, 256], F32, isOutput=False)
    w34T_d = nc.declare_dram_parameter("w34T", [128, 3], F32, isOutput=False)
    if with_bias:
        p1t_d = nc.declare_dram_parameter("p1t", [128, 2 * N], F32, isOutput=False)
        p2t_d = nc.declare_dram_parameter("p2t", [128, N], F32, isOutput=False)
        cpt_d = nc.declare_dram_parameter("cpt", [3 * IPC, N], F32, isOutput=False)
    out_d = nc.declare_dram_parameter("outp", [3 * IPC, N], F32, isOutput=True)

    with TileContext(nc) as tc:
        with (
            tc.tile_pool(name="const", bufs=1) as cpool,
            tc.tile_pool(name="acts", bufs=2) as apool,
            tc.tile_pool(name="psf", bufs=2, space="PSUM") as psf,
            tc.tile_pool(name="psa", bufs=3, space="PSUM") as psa,
        ):
            anT = cpool.tile([128, KT * N], F32)  # [p, k*768 + j]
            nc.sync.dma_start(
                anT[:, :].rearrange("p (k j) -> p k j", j=N),
                anT_d[:, :].rearrange("(k p) j -> p k j", p=128))
            a2T = cpool.tile([128, KT * N], F32)
            nc.sync.dma_start(
                a2T[:, :].rearrange("p (k j) -> p k j", j=N),
                a2T_d[:, :].rearrange("(k p) j -> p k j", p=128))
            w1T = cpool.tile([6, 256], F32)
            nc.sync.dma_start(w1T[:, :], w1T_d[:, :])
            w2Tp = cpool.tile([128, 256], F32)
            nc.sync.dma_start(w2Tp[:, :], w2Tp_d[:, :])
            w34T = cpool.tile([128, 3], F32)
            nc.sync.dma_start(w34T[:, :], w34T_d[:, :])
            if with_bias:
                p1t = cpool.tile([128, 2 * N], F32)
                nc.sync.dma_start(p1t[:, :], p1t_d[:, :])
                p2t = cpool.tile([128, N], F32)
                nc.sync.dma_start(p2t[:, :], p2t_d[:, :])
                cpt = cpool.tile([3 * IPC, N], F32)
                nc.sync.dma_start(cpt[:, :], cpt_d[:, :])

            # Z34 for all items: [p, k*3*IPC + it*3 + f]
            z34 = cpool.tile([128, KT * 3 * IPC], F32)

            for it in range(IPC):
                xT = apool.tile([6, N], F32, tag="xT")
                nc.sync.dma_start(xT[:, :], xT_d[it])

                # feat1: Z1[node, fo] = sum_fi xT[fi, node] * W1T[fi, fo]
                z1 = apool.tile([128, KT * 256], F32, tag="z1")  # [p, m*256 + fo]
                for m in range(KT):
                    ps = psf.tile([128, 256], F32, tag="feat")
                    nc.tensor.matmul(
                        ps[:, :], xT[:, m * 128:(m + 1) * 128], w1T[:, :],
                        start=True, stop=True,
                    )
                    nc.vector.tensor_copy(z1[:, m * 256:(m + 1) * 256], ps[:, :])

                # agg1: H1t[f, j] = relu(sum_k Z1[k, f] * AnT[k, j] (+ s x b1))
                h1t = apool.tile([128, 2 * N], F32, tag="h1t")  # [fi, fh*768 + n]
                for fh in range(2):
                    for ns in range(2):
                        ps = psa.tile([128, 384], F32, tag="agg")
                        for k in range(KT):
                            nc.tensor.matmul(
                                ps[:, :],
                                z1[:, k * 256 + fh * 128: k * 256 + fh * 128 + 128],
                                anT[:, k * N + ns * 384: k * N + ns * 384 + 384],
                                start=(k == 0), stop=(k == KT - 1),
                            )
                        dst = h1t[:, fh * N + ns * 384: fh * N + ns * 384 + 384]
                        if with_bias:
                            nc.vector.tensor_tensor(
                                dst, ps[:, :],
                                p1t[:, fh * N + ns * 384: fh * N + ns * 384 + 384],
                                op=mybir.AluOpType.add,
                            )
                            nc.scalar.activation(dst, dst, RELU)
                        else:
                            nc.scalar.activation(dst, ps[:, :], RELU)

                # feat2: Z2[node, fo] = sum_fi H1t[fi, node] * W2T[fi, fo]
                z2 = apool.tile([128, KT * 128], F32, tag="z2")  # [p, m*128 + fo]
                for m in range(KT):
                    ps = psf.tile([128, 128], F32, tag="feat")
                    for kh in range(2):
                        nc.tensor.matmul(
                            ps[:, :],
                            h1t[:, kh * N + m * 128: kh * N + m * 128 + 128],
                            w2Tp[:, kh * 128:(kh + 1) * 128],
                            start=(kh == 0), stop=(kh == 1),
                        )
                    nc.vector.tensor_copy(z2[:, m * 128:(m + 1) * 128], ps[:, :])

                # agg2 + relu -> H2t (feature-major, 128 x 768)
                h2t = apool.tile([128, N], F32, tag="h2t")
                for ns in range(2):
                    ps = psa.tile([128, 384], F32, tag="agg")
                    for k in range(KT):
                        nc.tensor.matmul(
                            ps[:, :],
                            z2[:, k * 128:(k + 1) * 128],
                            anT[:, k * N + ns * 384: k * N + ns * 384 + 384],
                            start=(k == 0), stop=(k == KT - 1),
                        )
                    dst = h2t[:, ns * 384: ns * 384 + 384]
                    if with_bias:
                        nc.vector.tensor_tensor(
                            dst, ps[:, :], p2t[:, ns * 384: ns * 384 + 384],
                            op=mybir.AluOpType.add,
                        )
                        nc.scalar.activation(dst, dst, RELU)
                    else:
                        nc.scalar.activation(dst, ps[:, :], RELU)

                # feat34: Z34[node, f] = sum_fi H2t[fi, node] * W34T[fi, f]
                for m in range(KT):
                    ps = psf.tile([128, 3], F32, tag="feat")
                    nc.tensor.matmul(
                        ps[:, :], h2t[:, m * 128:(m + 1) * 128], w34T[:, :],
                        start=True, stop=True,
                    )
                    base = m * 3 * IPC + it * 3
                    nc.vector.tensor_copy(z34[:, base: base + 3], ps[:, :])

            # final aggregation with A2 for all items at once
            outT = cpool.tile([3 * IPC, N], F32)
            for ns in range(2):
                ps = psa.tile([3 * IPC, 384], F32, tag="agg")
                for k in range(KT):
                    nc.tensor.matmul(
                        ps[:, :],
                        z34[:, k * 3 * IPC:(k + 1) * 3 * IPC],
                        a2T[:, k * N + ns * 384: k * N + ns * 384 + 384],
                        start=(k == 0), stop=(k == KT - 1),
                    )
                dst = outT[:, ns * 384: ns * 384 + 384]
                if with_bias:
                    nc.vector.tensor_tensor(
                        dst, ps[:, :], cpt[:, ns * 384: ns * 384 + 384],
                        op=mybir.AluOpType.add,
                    )
                else:
                    nc.vector.tensor_copy(dst, ps[:, :])
            nc.sync.dma_start(out_d[:, :], outT[:, :])

    return nc


MULT = mybir.AluOpType.mult
ADD = mybir.AluOpType.add


def _build_program_v2(ent_l1, ent_mid, ent_out):
    """Fast path. All activations feature-major; PE does weights-stationary
    feature matmuls; aggregation with A_norm runs on the vector engine:
      A_norm = T + E,  T[n,m] = d[n]*d[m] for |n-m|<=1,  E sparse.
    The trailing d-scale of each aggregation is deferred through the next
    feature matmul / relu (a per-node column scale commutes with both, d>=0),
    so each aggregation is 3 full DVE passes:
      U = plane .* Z;  S[n] = U[n-1]+U[n]+U[n+1];  plus sparse E ops.
    ent_*: (j, k, c) lists with coefficients pre-adjusted for the deferral.
    L2..L4 are emitted per 4-item group so PE/DVE/ACT pipeline across groups.
    """
    nc = bass.Bass()
    W = IPC * N  # 6144
    COPYF = mybir.ActivationFunctionType.Copy

    xpk_d = nc.declare_dram_parameter("xpk", [2, 128, N], F32, isOutput=False)
    dpl_d = nc.declare_dram_parameter("dpl", [128, N], F32, isOutput=False)
    dp2_d = nc.declare_dram_parameter("dp2", [128, N], F32, isOutput=False)
    w1rep_d = nc.declare_dram_parameter("w1rep", [128, 256], F32, isOutput=False)
    w2Tp_d = nc.declare_dram_parameter("w2Tp", [128, 256], F32, isOutput=False)
    w3T_d = nc.declare_dram_parameter("w3T", [128, 128], F32, isOutput=False)
    w4T_d = nc.declare_dram_parameter("w4T", [128, 3], F32, isOutput=False)
    out_d = nc.declare_dram_parameter("outp", [2, 128, N], F32, isOutput=True)

    with TileContext(nc) as tc:
        with (
            tc.tile_pool(name="const", bufs=1) as cpool,
            tc.tile_pool(name="acts", bufs=1) as apool,
            tc.tile_pool(name="grp", bufs=2) as gpool,
            tc.tile_pool(name="ps1", bufs=2, space="PSUM") as ps1,
            tc.tile_pool(name="ps2", bufs=4, space="PSUM") as ps2,
            tc.tile_pool(name="ps4", bufs=2, space="PSUM") as ps4,
        ):
            dpl = cpool.tile([128, N], F32)
            nc.sync.dma_start(dpl[:, :], dpl_d[:, :])
            dp2 = cpool.tile([128, N], F32)
            nc.sync.dma_start(dp2[:, :], dp2_d[:, :])
            w1rep = cpool.tile([128, 256], F32)
            nc.sync.dma_start(w1rep[:, :], w1rep_d[:, :])
            w2Tp = cpool.tile([128, 256], F32)
            nc.sync.dma_start(w2Tp[:, :], w2Tp_d[:, :])
            w3T = cpool.tile([128, 128], F32)
            nc.sync.dma_start(w3T[:, :], w3T_d[:, :])
            w4T = cpool.tile([128, 3], F32)
            nc.sync.dma_start(w4T[:, :], w4T_d[:, :])

            def tri(Z, H, U, zb, b, wid, plane, P=128, ubase=None):
                """S-part of one aggregation on flat tiles: windows
                Z[:, zb:], H[:, b:], U[:, u:] of width wid.
                U = plane.*Z;  H[n] = U[n-1]+U[n]+U[n+1] (in-window)."""
                u = b if ubase is None else ubase
                dv = plane[0:P, 0:wid]
                nc.vector.tensor_mul(U[0:P, u:u + wid], dv, Z[0:P, zb:zb + wid])
                nc.vector.tensor_add(H[0:P, b + 1:b + wid],
                                     U[0:P, u + 1:u + wid],
                                     U[0:P, u:u + wid - 1])
                nc.vector.tensor_copy(H[0:P, b:b + 1], U[0:P, u:u + 1])
                nc.vector.tensor_add(H[0:P, b:b + wid - 1],
                                     H[0:P, b:b + wid - 1],
                                     U[0:P, u + 1:u + wid])

            def ent_cols(Z, H, ents, zb=0, b=0, P=128):
                for (j, k, c) in ents:
                    nc.vector.scalar_tensor_tensor(
                        H[0:P, b + j:b + j + 1], Z[0:P, zb + k:zb + k + 1],
                        float(c), H[0:P, b + j:b + j + 1], op0=MULT, op1=ADD)

            def ent_group(Z, H, ents):
                zv = Z[:, :].rearrange("p (i n) -> p i n", n=N)
                hv = H[:, :].rearrange("p (i n) -> p i n", n=N)
                for (j, k, c) in ents:
                    nc.vector.scalar_tensor_tensor(
                        hv[:, :, j:j + 1], zv[:, :, k:k + 1], float(c),
                        hv[:, :, j:j + 1], op0=MULT, op1=ADD)

            # ---- L1: G' = (unscaled) aggregation of x;  true G = d .* G' ----
            G = []
            for g in range(2):
                Xg = apool.tile([128, N], F32, tag=f"xg{g}")
                nc.sync.dma_start(Xg[:, :], xpk_d[g])
                Gg = apool.tile([128, N], F32, tag=f"gg{g}")
                Ug = apool.tile([128, N], F32, tag=f"ug{g}")
                tri(Xg, Gg, Ug, 0, 0, N, dpl)
                ent_cols(Xg, Gg, ent_l1)
                G.append(Gg)

            # ---- feat1 (K=6 row-packed, weights stationary) + relu ----
            h1a = apool.tile([128, W], F32, tag="h1a")
            h1b = apool.tile([128, W], F32, tag="h1b")
            H1 = [h1a, h1b]
            for g in range(2):
                for half in range(2):
                    for ns in range(2):
                        for j in range(4):
                            it = g * 4 + j
                            ps = ps1.tile([128, 384], F32, tag="f1")
                            nc.tensor.matmul(
                                ps[:, :],
                                w1rep[32 * j:32 * j + 6,
                                      half * 128:(half + 1) * 128],
                                G[g][32 * j:32 * j + 6,
                                     ns * 384:(ns + 1) * 384],
                                start=True, stop=True,
                                tile_position=(32 * j, 0))
                            nc.scalar.activation(
                                H1[half][:, it * N + ns * 384:
                                         it * N + (ns + 1) * 384],
                                ps[:, :], RELU)

            def feat2(gi, Z2g):
                for j4 in range(4):
                    it = gi * 4 + j4
                    for cs in range(2):
                        ps = ps2.tile([128, 384], F32, tag="f2")
                        for kh in range(2):
                            nc.tensor.matmul(
                                ps[:, :], w2Tp[:, kh * 128:(kh + 1) * 128],
                                H1[kh][:, it * N + cs * 384:
                                       it * N + (cs + 1) * 384],
                                start=(kh == 0), stop=(kh == 1))
                        nc.scalar.activation(
                            Z2g[:, j4 * N + cs * 384: j4 * N + (cs + 1) * 384],
                            ps[:, :], COPYF)

            def agg(Zg, Hg, relu):
                for j4 in range(4):
                    Ut = gpool.tile([128, N], F32, tag="tagU")
                    tri(Zg, Hg, Ut, j4 * N, j4 * N, N, dp2, ubase=0)
                ent_group(Zg, Hg, ent_mid)
                if relu:
                    nc.scalar.activation(Hg[:, :], Hg[:, :], RELU)

            def feat3(gi, H2g, Z3g):
                for j4 in range(4):
                    w = j4 * N
                    for cs in range(2):
                        ps = ps2.tile([128, 384], F32, tag="f2")
                        nc.tensor.matmul(
                            ps[:, :], w3T[:, :],
                            H2g[:, w + cs * 384: w + (cs + 1) * 384],
                            start=True, stop=True)
                        nc.scalar.activation(
                            Z3g[:, w + cs * 384: w + (cs + 1) * 384],
                            ps[:, :], COPYF)

            def feat4_agg4(gi, H3g):
                G4 = gpool.tile([128, N], F32, tag="g4")
                for ns in range(2):
                    ps = ps4.tile([128, 384], F32, tag="f4")
                    for j in range(4):
                        nc.tensor.matmul(
                            ps[32 * j:32 * j + 3, :], w4T[:, :],
                            H3g[:, j * N + ns * 384: j * N + (ns + 1) * 384],
                            start=True, stop=True,
                            tile_position=(0, 32 * j))
                    nc.vector.tensor_copy(
                        G4[:, ns * 384:(ns + 1) * 384], ps[:, :])
                U4 = gpool.tile([128, N], F32, tag="tagU")
                O4 = gpool.tile([128, N], F32, tag="o4")
                tri(G4, O4, U4, 0, 0, N, dp2, ubase=0)
                nc.vector.tensor_mul(O4[:, :], dpl[:, :], O4[:, :])
                ent_cols(G4, O4, ent_out)
                nc.sync.dma_start(out_d[gi], O4[:, :])

            # phase-major emission: the PE stream never blocks on a group's
            # aggregation — the other group's feature matmuls come first
            Z2a = gpool.tile([128, 4 * N], F32, tag="tagZ")
            Z2b = gpool.tile([128, 4 * N], F32, tag="tagZ")
            feat2(0, Z2a)
            feat2(1, Z2b)
            H2a = gpool.tile([128, 4 * N], F32, tag="tagH")
            H2b = gpool.tile([128, 4 * N], F32, tag="tagH")
            agg(Z2a, H2a, relu=True)
            agg(Z2b, H2b, relu=True)
            Z3a = gpool.tile([128, 4 * N], F32, tag="tagZ")
            Z3b = gpool.tile([128, 4 * N], F32, tag="tagZ")
            feat3(0, H2a, Z3a)
            feat3(1, H2b, Z3b)
            H3a = gpool.tile([128, 4 * N], F32, tag="tagH")
            H3b = gpool.tile([128, 4 * N], F32, tag="tagH")
            agg(Z3a, H3a, relu=False)
            agg(Z3b, H3b, relu=False)
            feat4_agg4(0, H3a)
            feat4_agg4(1, H3b)

    return nc


def kernel(x, inputs, adjacency, W1, b1, W2, b2, W3, b3, W4, b4,
           parent_sel, child1_sel, child2_sel):
    global LAST_RUN_INFO
    x = np.asarray(x, np.float32)
    inp = np.asarray(inputs, np.float32)
    A = np.asarray(adjacency, np.float32)
    W1 = np.asarray(W1, np.float32); b1 = np.asarray(b1, np.float32)
    W2 = np.asarray(W2, np.float32); b2 = np.asarray(b2, np.float32)
    W3 = np.asarray(W3, np.float32); b3 = np.asarray(b3, np.float32)
    W4 = np.asarray(W4, np.float32); b4 = np.asarray(b4, np.float32)
    parent_sel = np.asarray(parent_sel, np.int64)
    child1_sel = np.asarray(child1_sel, np.int64)
    child2_sel = np.asarray(child2_sel, np.int64)

    # ---- host prep (replicated constants + layout marshaling) ----
    # clamp rows in global node index space
    clamp_rows = np.concatenate([
        parent_sel, NV + child1_sel, 2 * NV + child2_sel,
    ]).astype(np.int64)

    x0 = x.copy()
    x0[:, clamp_rows, 0:3] = inp[:, clamp_rows, :]

    deg = A.sum(axis=-1)
    deg_safe = np.where(deg == 0, np.float32(1.0), deg)
    d = np.where(deg == 0, np.float32(0.0), deg_safe ** np.float32(-0.5)).astype(np.float32)
    A_norm = (A * d[:, None] * d[None, :]).astype(np.float32)
    AnT = np.ascontiguousarray(A_norm.T)
    A2T = np.ascontiguousarray((A_norm @ A_norm).T.astype(np.float32))

    W1T = np.ascontiguousarray(W1.T)                       # (6, 256)
    W2Tp = np.ascontiguousarray(                           # (128, 256): [p, kh*128+f]
        W2.T.reshape(2, 128, 128).transpose(1, 0, 2).reshape(128, 256))
    W34T = np.ascontiguousarray(W3.T @ W4.T)               # (128, 3)

    with_bias = bool(np.any(b1) or np.any(b2) or np.any(b3) or np.any(b4))
    extra = {}
    if with_bias:
        s = A_norm.sum(axis=1).astype(np.float32)          # A_norm @ 1
        s2 = (A_norm @ s).astype(np.float32)
        # P1t[fi, fh*768 + n] = b1[fh*128+fi] * s[n]
        p1t = np.einsum('f,n->fn', b1, s).astype(np.float32)        # (256, 768)
        p1t = p1t.reshape(2, 128, N).transpose(1, 0, 2).reshape(128, 2 * N)
        p2t = np.einsum('f,n->fn', b2, s).astype(np.float32)        # (128, 768)
        cp = (np.einsum('f,n->fn', W4 @ b3, s2) +
              np.einsum('f,n->fn', b4, s)).astype(np.float32)       # (3, 768)
        cpt = np.tile(cp, (IPC, 1)).astype(np.float32)              # (24, 768)
        extra = {"p1t": np.ascontiguousarray(p1t),
                 "p2t": np.ascontiguousarray(p2t),
                 "cpt": np.ascontiguousarray(cpt)}

    # sparse residual of A_norm vs the tridiagonal d-outer-product model
    E = A_norm.copy()
    idx = np.arange(N)
    for o in (-1, 0, 1):
        n = idx[max(0, -o):N - max(0, o)]
        E[n, n + o] -= (d[n] * d[n + o]).astype(np.float32)
    nz = np.argwhere(E != 0)
    entries = [(int(j), int(k), float(E[j, k])) for j, k in nz]

    use_v2 = (not with_bias) and len(entries) <= 96

    if use_v2:
        # item-packed inputs: 2 groups of 4 items at partition stride 32
        xpk = np.zeros((NCORES, 2, 128, N), np.float32)
        for c in range(NCORES):
            for g in range(2):
                for j in range(4):
                    xpk[c, g, 32 * j:32 * j + 6, :] = \
                        x0[c * IPC + g * 4 + j].T
        dpl = np.ascontiguousarray(
            np.broadcast_to(d, (128, N)).astype(np.float32))
        dp2 = np.ascontiguousarray(
            np.broadcast_to((d * d).astype(np.float32), (128, N)))
        w1rep = np.zeros((128, 256), np.float32)
        for j in range(4):
            w1rep[32 * j:32 * j + 6, :] = W1T
        w3T = np.ascontiguousarray(W3.T)
        w4T = np.ascontiguousarray(W4.T)

        # entry coefficients adjusted for the deferred d-scale
        dj = np.where(d == 0, np.float32(1.0), d)
        ent_l1 = [(j, k, c / float(dj[j])) for (j, k, c) in entries]
        ent_mid = [(j, k, c * float(d[k]) / float(dj[j]))
                   for (j, k, c) in entries]
        ent_out = [(j, k, c * float(d[k])) for (j, k, c) in entries]

        nc = _build_program_v2(ent_l1, ent_mid, ent_out)
        _split_multi_waits(nc)
        in_maps = [{
            "xpk": xpk[c], "dpl": dpl, "dp2": dp2, "w1rep": w1rep,
            "w2Tp": W2Tp, "w3T": w3T, "w4T": w4T,
        } for c in range(NCORES)]
    else:
        # per-core input shards: xT[core][it] = x0[core*IPC+it].T  (6, 768)
        xT_all = np.ascontiguousarray(
            x0.transpose(0, 2, 1).reshape(NCORES, IPC, 6, N))

        nc = _build_program(with_bias)
        _split_multi_waits(nc)

        in_maps = []
        for c in range(NCORES):
            m = {
                "xT": xT_all[c],
                "anT": AnT,
                "a2T": A2T,
                "w1T": W1T,
                "w2Tp": W2Tp,
                "w34T": W34T,
            }
            m.update(extra)
            in_maps.append(m)

    trace = os.environ.get("KERNEL_TRACE", "") == "1"
    res = run_bass_kernel_spmd(nc, in_maps, list(range(NCORES)), trace=trace)

    LAST_RUN_INFO = {
        "exec_time_ns": res.exec_time_ns,
        "mean_exec_time_ns": res.mean_exec_time_ns,
        "max_exec_time_core_id": res.max_exec_time_core_id,
    }

    out = np.empty((B, N, 3), np.float32)
    for c in range(NCORES):
        o = res.results[c]["outp"]
        if use_v2:  # (2, 128, 768), item g*4+j at partitions 32j..32j+3
            for g in range(2):
                for j in range(4):
                    out[c * IPC + g * 4 + j] = o[g, 32 * j:32 * j + 3, :].T
        else:       # (24, 768)
            for it in range(IPC):
                out[c * IPC + it] = o[it * 3:(it + 1) * 3, :].T
    # output clamp
    out[:, clamp_rows, :] = inp[:, clamp_rows, :]
    return out


# revision 19
# speedup vs baseline: 2.9053x; 1.1929x over previous
"""Trainium2 Bass kernel for BatchedGNNModel (4-layer GCN over 3-rod chain graph).

Contract: kernel(**inputs) takes FULL unsharded inputs (as produced by
setup_inputs) and returns the FULL (64, 768, 3) float32 output.

Strategy (pure data-parallel over batch, 8 NeuronCores):
  - Host: normalize adjacency (A_norm), precompute A_norm^T, (A_norm@A_norm)^T,
    transposed weights, clamp input positions, pack per-core shards.
  - Device (identical SPMD program, different batch shard per core):
      per item:  x^T --feat1(K=6)--> Z1 (node-major)
                 --agg1 (A_norm^T, accumulate over node K-tiles)--> H1 (feat-major, relu)
                 --feat2--> Z2 --agg2--> H2 (relu)
                 --feat34 (W3^T@W4^T fused; L3/L4 have no relu between)--> Z34
      then one packed agg over A2 = A_norm@A_norm for all items -> out^T.
  - Host: gather per-core outputs, transpose, apply output clamp.

Layouts alternate feature-major <-> node-major so no on-device transposes are
needed: feature matmuls contract the feature dim (activations stationary),
aggregations contract the node dim (activations stationary, A^T moving).
"""

import os
import sys

import numpy as np

sys.path.insert(0, "/opt/trn_rl_repo")

import concourse.bass as bass
import concourse.mybir as mybir
import concourse.tile as _tile_mod
from concourse.tile import TileContext
from concourse.vector_clock import ScopedClock
from concourse.bass_utils import run_bass_kernel_spmd


def _patched_drain_and_barrier(self, tick_clock, wait_clock):
    """The nix walrus in this image only supports one sync-wait slot on a
    Drain; Tile's kernel-tail drain carries one wait per ticked semaphore.
    Split the extra waits onto single-wait nops on the same (sync) engine —
    program order makes this equivalent before the all-engine barrier."""
    drain_inst = self.nc.sync.drain()
    wait_clock.add_sem_waits(
        drain_inst.ins, ScopedClock({None: tick_clock.global_clock}))
    waits = list(drain_inst.ins.sync_info.on_wait)
    if len(waits) > 1:
        drain_inst.ins.sync_info.on_wait = [waits[0]]
        for w in waits[1:]:
            import bass_rust
            nop = self.nc.sync.nop(nofuse=True)
            si = nop.ins.sync_info
            if si is None:
                nop.ins.sync_info = bass_rust.SyncInfo(on_wait=[w], on_update=[])
            else:
                si.on_wait = [w]
    self.nc.all_engine_barrier()
    assert self.sems is not None
    popped = self.nc._tile_sem_poison_stack.pop()
    assert popped is self._sem_poison
    self.nc.clear_and_free_semaphores(list(self.sems.allocated().values()))
    self.nc.all_engine_barrier()


_tile_mod.TileContext._drain_and_barrier = _patched_drain_and_barrier


def _split_multi_waits(nc):
    """This image's walrus supports a single sync-wait slot per instruction.
    Hoist all-but-one wait of any multi-wait instruction onto single-wait
    NoOps on the same engine, placed immediately before it (same per-engine
    program order => equivalent synchronization)."""
    for f in nc.m.functions:
        for bb in f.blocks:
            insts = list(bb.instructions)
            if not any(ins.sync_info and len(ins.sync_info.on_wait) > 1
                       for ins in insts):
                continue
            new = []
            for ins in insts:
                si = ins.sync_info
                if si is not None and len(si.on_wait) > 1:
                    waits = list(si.on_wait)
                    for w in waits[:-1]:
                        new.append(mybir.InstNoOp(
                            name=nc.get_next_instruction_name(),
                            sync_info=mybir.SyncInfo(on_wait=[w], on_update=[]),
                            bass_nofuse=True,
                            engine=ins.engine,
                        ))
                    si.on_wait = [waits[-1]]
                new.append(ins)
            bb.instructions = new


def _ensure_ntff_hook():
    """The agent image's antenv lacks axon_hooks; bass_utils imports it when
    trace=True. Install a shim and, if possible, the real ctypes profiler."""
    import types
    try:
        import antenv.axon_hooks  # noqa: F401
        return
    except Exception:
        pass
    try:
        import antenv
        mod = types.ModuleType("antenv.axon_hooks")
        state = {"h": None}
        mod.set_axon_ntff_profile_hook = lambda h: state.__setitem__("h", h)
        mod.get_axon_ntff_profile_hook = lambda: state["h"]
        sys.modules["antenv.axon_hooks"] = mod
        antenv.axon_hooks = mod
        try:
            from trn_agent_boot.trn_boot import _ntff_profile_via_ctypes
            mod.set_axon_ntff_profile_hook(
                _ntff_profile_via_ctypes("/opt/axon/libaxon_pjrt.so"))
        except Exception:
            pass
    except Exception:
        pass


_ensure_ntff_hook()

F32 = mybir.dt.float32
RELU = mybir.ActivationFunctionType.Relu

B = 64
NV = 256
N = 3 * NV  # 768
NCORES = 8
IPC = B // NCORES  # 8 items per core
KT = N // 128      # 6 node K-tiles

LAST_RUN_INFO = {}


def _build_program(with_bias: bool):
    nc = bass.Bass()

    xT_d = nc.declare_dram_parameter("xT", [IPC, 6, N], F32, isOutput=False)
    anT_d = nc.declare_dram_parameter("anT", [N, N], F32, isOutput=False)
    a2T_d = nc.declare_dram_parameter("a2T", [N, N], F32, isOutput=False)
    w1T_d = nc.declare_dram_parameter("w1T", [6, 256], F32, isOutput=False)
    w2Tp_d = nc.declare_dram_parameter("w2Tp", [128, 256], F32, isOutput=False)
    w34T_d = nc.declare_dram_parameter("w34T", [128, 3], F32, isOutput=False)
    if with_bias:
        p1t_d = nc.declare_dram_parameter("p1t", [128, 2 * N], F32, isOutput=False)
        p2t_d = nc.declare_dram_parameter("p2t", [128, N], F32, isOutput=False)
        cpt_d = nc.declare_dram_parameter("cpt", [3 * IPC, N], F32, isOutput=False)
    out_d = nc.declare_dram_parameter("outp", [3 * IPC, N], F32, isOutput=True)

    with TileContext(nc) as tc:
        with (
            tc.tile_pool(name="const", bufs=1) as cpool,
            tc.tile_pool(name="acts", bufs=2) as apool,
            tc.tile_pool(name="psf", bufs=2, space="PSUM") as psf,
            tc.tile_pool(name="psa", bufs=3, space="PSUM") as psa,
        ):
            anT = cpool.tile([128, KT * N], F32)  # [p, k*768 + j]
            nc.sync.dma_start(
                anT[:, :].rearrange("p (k j) -> p k j", j=N),
                anT_d[:, :].rearrange("(k p) j -> p k j", p=128))
            a2T = cpool.tile([128, KT * N], F32)
            nc.sync.dma_start(
                a2T[:, :].rearrange("p (k j) -> p k j", j=N),
                a2T_d[:, :].rearrange("(k p) j -> p k j", p=128))
            w1T = cpool.tile([6, 256], F32)
            nc.sync.dma_start(w1T[:, :], w1T_d[:, :])
            w2Tp = cpool.tile([128, 256], F32R)
            nc.sync.dma_start(w2Tp[:, :], w2Tp_d[:, :])
            w34T = cpool.tile([128, 3], F32)
            nc.sync.dma_start(w34T[:, :], w34T_d[:, :])
            if with_bias:
                p1t = cpool.tile([128, 2 * N], F32)
                nc.sync.dma_start(p1t[:, :], p1t_d[:, :])
                p2t = cpool.tile([128, N], F32)
                nc.sync.dma_start(p2t[:, :], p2t_d[:, :])
                cpt = cpool.tile([3 * IPC, N], F32)
                nc.sync.dma_start(cpt[:, :], cpt_d[:, :])

            # Z34 for all items: [p, k*3*IPC + it*3 + f]
            z34 = cpool.tile([128, KT * 3 * IPC], F32)

            for it in range(IPC):
                xT = apool.tile([6, N], F32, tag="xT")
                nc.sync.dma_start(xT[:, :], xT_d[it])

                # feat1: Z1[node, fo] = sum_fi xT[fi, node] * W1T[fi, fo]
                z1 = apool.tile([128, KT * 256], F32, tag="z1")  # [p, m*256 + fo]
                for m in range(KT):
                    ps = psf.tile([128, 256], F32, tag="feat")
                    nc.tensor.matmul(
                        ps[:, :], xT[:, m * 128:(m + 1) * 128], w1T[:, :],
                        start=True, stop=True,
                    )
                    nc.vector.tensor_copy(z1[:, m * 256:(m + 1) * 256], ps[:, :])

                # agg1: H1t[f, j] = relu(sum_k Z1[k, f] * AnT[k, j] (+ s x b1))
                h1t = apool.tile([128, 2 * N], F32, tag="h1t")  # [fi, fh*768 + n]
                for fh in range(2):
                    for ns in range(2):
                        ps = psa.tile([128, 384], F32, tag="agg")
                        for k in range(KT):
                            nc.tensor.matmul(
                                ps[:, :],
                                z1[:, k * 256 + fh * 128: k * 256 + fh * 128 + 128],
                                anT[:, k * N + ns * 384: k * N + ns * 384 + 384],
                                start=(k == 0), stop=(k == KT - 1),
                            )
                        dst = h1t[:, fh * N + ns * 384: fh * N + ns * 384 + 384]
                        if with_bias:
                            nc.vector.tensor_tensor(
                                dst, ps[:, :],
                                p1t[:, fh * N + ns * 384: fh * N + ns * 384 + 384],
                                op=mybir.AluOpType.add,
                            )
                            nc.scalar.activation(dst, dst, RELU)
                        else:
                            nc.scalar.activation(dst, ps[:, :], RELU)

                # feat2: Z2[node, fo] = sum_fi H1t[fi, node] * W2T[fi, fo]
                z2 = apool.tile([128, KT * 128], F32, tag="z2")  # [p, m*128 + fo]
                for m in range(KT):
                    ps = psf.tile([128, 128], F32, tag="feat")
                    for kh in range(2):
                        nc.tensor.matmul(
                            ps[:, :],
                            h1t[:, kh * N + m * 128: kh * N + m * 128 + 128],
                            w2Tp[:, kh * 128:(kh + 1) * 128],
                            start=(kh == 0), stop=(kh == 1),
                        )
                    nc.vector.tensor_copy(z2[:, m * 128:(m + 1) * 128], ps[:, :])

                # agg2 + relu -> H2t (feature-major, 128 x 768)
                h2t = apool.tile([128, N], F32, tag="h2t")
                for ns in range(2):
                    ps = psa.tile([128, 384], F32, tag="agg")
                    for k in range(KT):
                        nc.tensor.matmul(
                            ps[:, :],
                            z2[:, k * 128:(k + 1) * 128],
                            anT[:, k * N + ns * 384: k * N + ns * 384 + 384],
                            start=(k == 0), stop=(k == KT - 1),
                        )
                    dst = h2t[:, ns * 384: ns * 384 + 384]
                    if with_bias:
                        nc.vector.tensor_tensor(
                            dst, ps[:, :], p2t[:, ns * 384: ns * 384 + 384],
                            op=mybir.AluOpType.add,
                        )
                        nc.scalar.activation(dst, dst, RELU)
                    else:
                        nc.scalar.activation(dst, ps[:, :], RELU)

                # feat34: Z34[node, f] = sum_fi H2t[fi, node] * W34T[fi, f]
                for m in range(KT):
                    ps = psf.tile([128, 3], F32, tag="feat")
                    nc.tensor.matmul(
                        ps[:, :], h2t[:, m * 128:(m + 1) * 128], w34T[:, :],
                        start=True, stop=True,
                    )
                    base = m * 3 * IPC + it * 3
                    nc.vector.tensor_copy(z34[:, base: base + 3], ps[:, :])

            # final aggregation with A2 for all items at once
            outT = cpool.tile([3 * IPC, N], F32)
            for ns in range(2):
                ps = psa.tile([3 * IPC, 384], F32, tag="agg")
                for k in range(KT):
                    nc.tensor.matmul(
                        ps[:, :],
                        z34[:, k * 3 * IPC:(k + 1) * 3 * IPC],
                        a2T[:, k * N + ns * 384: k * N + ns * 384 + 384],
                        start=(k == 0), stop=(k == KT - 1),
                    )
                dst = outT[:, ns * 384: ns * 384 + 384]
                if with_bias:
                    nc.vector.tensor_tensor(
                        dst, ps[:, :], cpt[:, ns * 384: ns * 384 + 384],
                        op=mybir.AluOpType.add,
                    )
                else:
                    nc.vector.tensor_copy(dst, ps[:, :])
            nc.sync.dma_start(out_d[:, :], outT[:, :])

    return nc


MULT = mybir.AluOpType.mult
ADD = mybir.AluOpType.add


def _build_program_v2(ent_l1, ent_mid, ent_out):
    """Fast path. All activations feature-major; PE does weights-stationary
    feature matmuls; aggregation with A_norm runs on the vector engine:
      A_norm = T + E,  T[n,m] = d[n]*d[m] for |n-m|<=1,  E sparse.
    The trailing d-scale of each aggregation is deferred through the next
    feature matmul / relu (a per-node column scale commutes with both, d>=0),
    so each aggregation is 3 full DVE passes:
      U = plane .* Z;  S[n] = U[n-1]+U[n]+U[n+1];  plus sparse E ops.
    ent_*: (j, k, c) lists with coefficients pre-adjusted for the deferral.
    L2..L4 are emitted per 4-item group so PE/DVE/ACT pipeline across groups.
    """
    nc = bass.Bass()
    W = IPC * N  # 6144
    COPYF = mybir.ActivationFunctionType.Copy
    F32R = mybir.dt.float32r  # single-pass fp32 matmul mode

    xpk_d = nc.declare_dram_parameter("xpk", [2, 128, N], F32, isOutput=False)
    dpl_d = nc.declare_dram_parameter("dpl", [128, N], F32, isOutput=False)
    dp2_d = nc.declare_dram_parameter("dp2", [128, N], F32, isOutput=False)
    w1rep_d = nc.declare_dram_parameter("w1rep", [128, 256], F32R, isOutput=False)
    w2Tp_d = nc.declare_dram_parameter("w2Tp", [128, 256], F32R, isOutput=False)
    w3T_d = nc.declare_dram_parameter("w3T", [128, 128], F32R, isOutput=False)
    w4T_d = nc.declare_dram_parameter("w4T", [128, 3], F32, isOutput=False)
    out_d = nc.declare_dram_parameter("outp", [2, 128, N], F32, isOutput=True)

    with TileContext(nc) as tc:
        with (
            tc.tile_pool(name="const", bufs=1) as cpool,
            tc.tile_pool(name="acts", bufs=1) as apool,
            tc.tile_pool(name="grp", bufs=2) as gpool,
            tc.tile_pool(name="ps1", bufs=2, space="PSUM") as ps1,
            tc.tile_pool(name="ps2", bufs=4, space="PSUM") as ps2,
            tc.tile_pool(name="ps4", bufs=2, space="PSUM") as ps4,
        ):
            dpl = cpool.tile([128, N], F32)
            nc.sync.dma_start(dpl[:, :], dpl_d[:, :])
            dp2 = cpool.tile([128, N], F32)
            nc.sync.dma_start(dp2[:, :], dp2_d[:, :])
            w1rep = cpool.tile([128, 256], F32R)
            nc.sync.dma_start(w1rep[:, :], w1rep_d[:, :])
            w2Tp = cpool.tile([128, 256], F32R)
            nc.sync.dma_start(w2Tp[:, :], w2Tp_d[:, :])
            w3T = cpool.tile([128, 128], F32R)
            nc.sync.dma_start(w3T[:, :], w3T_d[:, :])
            w4T = cpool.tile([128, 3], F32)
            nc.sync.dma_start(w4T[:, :], w4T_d[:, :])

            def tri(Z, H, U, zb, b, wid, plane, P=128, ubase=None):
                """S-part of one aggregation on flat tiles: windows
                Z[:, zb:], H[:, b:], U[:, u:] of width wid.
                U = plane.*Z;  H[n] = U[n-1]+U[n]+U[n+1] (in-window)."""
                u = b if ubase is None else ubase
                dv = plane[0:P, 0:wid]
                nc.vector.tensor_mul(U[0:P, u:u + wid], dv, Z[0:P, zb:zb + wid])
                nc.vector.tensor_add(H[0:P, b + 1:b + wid],
                                     U[0:P, u + 1:u + wid],
                                     U[0:P, u:u + wid - 1])
                nc.vector.tensor_copy(H[0:P, b:b + 1], U[0:P, u:u + 1])
                nc.vector.tensor_add(H[0:P, b:b + wid - 1],
                                     H[0:P, b:b + wid - 1],
                                     U[0:P, u + 1:u + wid])

            def ent_cols(Z, H, ents, zb=0, b=0, P=128):
                for (j, k, c) in ents:
                    nc.vector.scalar_tensor_tensor(
                        H[0:P, b + j:b + j + 1], Z[0:P, zb + k:zb + k + 1],
                        float(c), H[0:P, b + j:b + j + 1], op0=MULT, op1=ADD)

            def ent_group(Z, H, ents):
                zv = Z[:, :].rearrange("p (i n) -> p i n", n=N)
                hv = H[:, :].rearrange("p (i n) -> p i n", n=N)
                for (j, k, c) in ents:
                    nc.vector.scalar_tensor_tensor(
                        hv[:, :, j:j + 1], zv[:, :, k:k + 1], float(c),
                        hv[:, :, j:j + 1], op0=MULT, op1=ADD)

            # ---- L1: G' = (unscaled) aggregation of x;  true G = d .* G' ----
            G = []
            for g in range(2):
                Xg = apool.tile([128, N], F32, tag=f"xg{g}")
                nc.sync.dma_start(Xg[:, :], xpk_d[g])
                Gg = apool.tile([128, N], F32R, tag=f"gg{g}")
                Ug = apool.tile([128, N], F32, tag=f"ug{g}")
                tri(Xg, Gg, Ug, 0, 0, N, dpl)
                ent_cols(Xg, Gg, ent_l1)
                G.append(Gg)

            # ---- feat1 (K=6 row-packed, weights stationary) + relu ----
            h1a = apool.tile([128, W], F32R, tag="h1a")
            h1b = apool.tile([128, W], F32R, tag="h1b")
            H1 = [h1a, h1b]
            for g in range(2):
                for half in range(2):
                    for ns in range(2):
                        for j in range(4):
                            it = g * 4 + j
                            ps = ps1.tile([128, 384], F32, tag="f1")
                            nc.tensor.matmul(
                                ps[:, :],
                                w1rep[32 * j:32 * j + 6,
                                      half * 128:(half + 1) * 128],
                                G[g][32 * j:32 * j + 6,
                                     ns * 384:(ns + 1) * 384],
                                start=True, stop=True,
                                tile_position=(32 * j, 0))
                            nc.scalar.activation(
                                H1[half][:, it * N + ns * 384:
                                         it * N + (ns + 1) * 384],
                                ps[:, :], RELU)

            def feat2(gi, Z2g):
                for j4 in range(4):
                    it = gi * 4 + j4
                    for cs in range(2):
                        ps = ps2.tile([128, 384], F32, tag="f2")
                        for kh in range(2):
                            nc.tensor.matmul(
                                ps[:, :], w2Tp[:, kh * 128:(kh + 1) * 128],
                                H1[kh][:, it * N + cs * 384:
                                       it * N + (cs + 1) * 384],
                                start=(kh == 0), stop=(kh == 1))
                        nc.scalar.activation(
                            Z2g[:, j4 * N + cs * 384: j4 * N + (cs + 1) * 384],
                            ps[:, :], COPYF)

            def agg(Zg, Hg, relu):
                for j4 in range(4):
                    Ut = gpool.tile([128, N], F32, tag="tagU")
                    tri(Zg, Hg, Ut, j4 * N, j4 * N, N, dp2, ubase=0)
                ent_group(Zg, Hg, ent_mid)
                if relu:
                    nc.scalar.activation(Hg[:, :], Hg[:, :], RELU)

            def feat3(gi, H2g, Z3g):
                for j4 in range(4):
                    w = j4 * N
                    for cs in range(2):
                        ps = ps2.tile([128, 384], F32, tag="f2")
                        nc.tensor.matmul(
                            ps[:, :], w3T[:, :],
                            H2g[:, w + cs * 384: w + (cs + 1) * 384],
                            start=True, stop=True)
                        nc.scalar.activation(
                            Z3g[:, w + cs * 384: w + (cs + 1) * 384],
                            ps[:, :], COPYF)

            def feat4_agg4(gi, H3g):
                G4 = gpool.tile([128, N], F32, tag="g4")
                for ns in range(2):
                    ps = ps4.tile([128, 384], F32, tag="f4")
                    for j in range(4):
                        nc.tensor.matmul(
                            ps[32 * j:32 * j + 3, :], w4T[:, :],
                            H3g[:, j * N + ns * 384:
                                j * N + (ns + 1) * 384].bitcast(F32),
                            start=True, stop=True,
                            tile_position=(0, 32 * j))
                    nc.vector.tensor_copy(
                        G4[:, ns * 384:(ns + 1) * 384], ps[:, :])
                U4 = gpool.tile([128, N], F32, tag="tagU")
                O4 = gpool.tile([128, N], F32, tag="o4")
                tri(G4, O4, U4, 0, 0, N, dp2, ubase=0)
                nc.vector.tensor_mul(O4[:, :], dpl[:, :], O4[:, :])
                ent_cols(G4, O4, ent_out)
                nc.sync.dma_start(out_d[gi], O4[:, :])

            # phase-major emission: the PE stream never blocks on a group's
            # aggregation — the other group's feature matmuls come first
            Z2a = gpool.tile([128, 4 * N], F32, tag="tagZ")
            Z2b = gpool.tile([128, 4 * N], F32, tag="tagZ")
            feat2(0, Z2a)
            feat2(1, Z2b)
            H2a = gpool.tile([128, 4 * N], F32R, tag="tagH")
            H2b = gpool.tile([128, 4 * N], F32R, tag="tagH")
            agg(Z2a, H2a, relu=True)
            agg(Z2b, H2b, relu=True)
            Z3a = gpool.tile([128, 4 * N], F32, tag="tagZ")
            Z3b = gpool.tile([128, 4 * N], F32, tag="tagZ")
            feat3(0, H2a, Z3a)
            feat3(1, H2b, Z3b)
            H3a = gpool.tile([128, 4 * N], F32R, tag="tagH")
            H3b = gpool.tile([128, 4 * N], F32R, tag="tagH")
            agg(Z3a, H3a, relu=False)
            agg(Z3b, H3b, relu=False)
            feat4_agg4(0, H3a)
            feat4_agg4(1, H3b)

    return nc


def kernel(x, inputs, adjacency, W1, b1, W2, b2, W3, b3, W4, b4,
           parent_sel, child1_sel, child2_sel):
    global LAST_RUN_INFO
    x = np.asarray(x, np.float32)
    inp = np.asarray(inputs, np.float32)
    A = np.asarray(adjacency, np.float32)
    W1 = np.asarray(W1, np.float32); b1 = np.asarray(b1, np.float32)
    W2 = np.asarray(W2, np.float32); b2 = np.asarray(b2, np.float32)
    W3 = np.asarray(W3, np.float32); b3 = np.asarray(b3, np.float32)
    W4 = np.asarray(W4, np.float32); b4 = np.asarray(b4, np.float32)
    parent_sel = np.asarray(parent_sel, np.int64)
    child1_sel = np.asarray(child1_sel, np.int64)
    child2_sel = np.asarray(child2_sel, np.int64)

    # ---- host prep (replicated constants + layout marshaling) ----
    # clamp rows in global node index space
    clamp_rows = np.concatenate([
        parent_sel, NV + child1_sel, 2 * NV + child2_sel,
    ]).astype(np.int64)

    x0 = x.copy()
    x0[:, clamp_rows, 0:3] = inp[:, clamp_rows, :]

    deg = A.sum(axis=-1)
    deg_safe = np.where(deg == 0, np.float32(1.0), deg)
    d = np.where(deg == 0, np.float32(0.0), deg_safe ** np.float32(-0.5)).astype(np.float32)
    A_norm = (A * d[:, None] * d[None, :]).astype(np.float32)
    AnT = np.ascontiguousarray(A_norm.T)
    A2T = np.ascontiguousarray((A_norm @ A_norm).T.astype(np.float32))

    W1T = np.ascontiguousarray(W1.T)                       # (6, 256)
    W2Tp = np.ascontiguousarray(                           # (128, 256): [p, kh*128+f]
        W2.T.reshape(2, 128, 128).transpose(1, 0, 2).reshape(128, 256))
    W34T = np.ascontiguousarray(W3.T @ W4.T)               # (128, 3)

    with_bias = bool(np.any(b1) or np.any(b2) or np.any(b3) or np.any(b4))
    extra = {}
    if with_bias:
        s = A_norm.sum(axis=1).astype(np.float32)          # A_norm @ 1
        s2 = (A_norm @ s).astype(np.float32)
        # P1t[fi, fh*768 + n] = b1[fh*128+fi] * s[n]
        p1t = np.einsum('f,n->fn', b1, s).astype(np.float32)        # (256, 768)
        p1t = p1t.reshape(2, 128, N).transpose(1, 0, 2).reshape(128, 2 * N)
        p2t = np.einsum('f,n->fn', b2, s).astype(np.float32)        # (128, 768)
        cp = (np.einsum('f,n->fn', W4 @ b3, s2) +
              np.einsum('f,n->fn', b4, s)).astype(np.float32)       # (3, 768)
        cpt = np.tile(cp, (IPC, 1)).astype(np.float32)              # (24, 768)
        extra = {"p1t": np.ascontiguousarray(p1t),
                 "p2t": np.ascontiguousarray(p2t),
                 "cpt": np.ascontiguousarray(cpt)}

    # sparse residual of A_norm vs the tridiagonal d-outer-product model
    E = A_norm.copy()
    idx = np.arange(N)
    for o in (-1, 0, 1):
        n = idx[max(0, -o):N - max(0, o)]
        E[n, n + o] -= (d[n] * d[n + o]).astype(np.float32)
    nz = np.argwhere(E != 0)
    entries = [(int(j), int(k), float(E[j, k])) for j, k in nz]

    use_v2 = (not with_bias) and len(entries) <= 96

    if use_v2:
        # item-packed inputs: 2 groups of 4 items at partition stride 32
        xpk = np.zeros((NCORES, 2, 128, N), np.float32)
        for c in range(NCORES):
            for g in range(2):
                for j in range(4):
                    xpk[c, g, 32 * j:32 * j + 6, :] = \
                        x0[c * IPC + g * 4 + j].T
        dpl = np.ascontiguousarray(
            np.broadcast_to(d, (128, N)).astype(np.float32))
        dp2 = np.ascontiguousarray(
            np.broadcast_to((d * d).astype(np.float32), (128, N)))
        w1rep = np.zeros((128, 256), np.float32)
        for j in range(4):
            w1rep[32 * j:32 * j + 6, :] = W1T
        w3T = np.ascontiguousarray(W3.T)
        w4T = np.ascontiguousarray(W4.T)

        # entry coefficients adjusted for the deferred d-scale
        dj = np.where(d == 0, np.float32(1.0), d)
        ent_l1 = [(j, k, c / float(dj[j])) for (j, k, c) in entries]
        ent_mid = [(j, k, c * float(d[k]) / float(dj[j]))
                   for (j, k, c) in entries]
        ent_out = [(j, k, c * float(d[k])) for (j, k, c) in entries]

        nc = _build_program_v2(ent_l1, ent_mid, ent_out)
        _split_multi_waits(nc)
        in_maps = [{
            "xpk": xpk[c], "dpl": dpl, "dp2": dp2, "w1rep": w1rep,
            "w2Tp": W2Tp, "w3T": w3T, "w4T": w4T,
        } for c in range(NCORES)]
    else:
        # per-core input shards: xT[core][it] = x0[core*IPC+it].T  (6, 768)
        xT_all = np.ascontiguousarray(
            x0.transpose(0, 2, 1).reshape(NCORES, IPC, 6, N))

        nc = _build_program(with_bias)
        _split_multi_waits(nc)

        in_maps = []
        for c in range(NCORES):
            m = {
                "xT": xT_all[c],
                "anT": AnT,
                "a2T": A2T,
                "w1T": W1T,
                "w2Tp": W2Tp,
                "w34T": W34T,
            }
            m.update(extra)
            in_maps.append(m)

    trace = os.environ.get("KERNEL_TRACE", "") == "1"
    res = run_bass_kernel_spmd(nc, in_maps, list(range(NCORES)), trace=trace)

    LAST_RUN_INFO = {
        "exec_time_ns": res.exec_time_ns,
        "mean_exec_time_ns": res.mean_exec_time_ns,
        "max_exec_time_core_id": res.max_exec_time_core_id,
    }

    out = np.empty((B, N, 3), np.float32)
    for c in range(NCORES):
        o = res.results[c]["outp"]
        if use_v2:  # (2, 128, 768), item g*4+j at partitions 32j..32j+3
            for g in range(2):
                for j in range(4):
                    out[c * IPC + g * 4 + j] = o[g, 32 * j:32 * j + 3, :].T
        else:       # (24, 768)
            for it in range(IPC):
                out[c * IPC + it] = o[it * 3:(it + 1) * 3, :].T
    # output clamp
    out[:, clamp_rows, :] = inp[:, clamp_rows, :]
    return out
